# revision 41
# baseline (speedup 1.0000x reference)
"""Trainium2 Bass kernel for nn_EnhancedFlowLayer (topk_masking).

8 cores. Tokens on partitions (2 groups of 128); flow (i,j)-space sharded by i
across cores (64 i-rows -> 32768 elems/token/core). flow is rematerialized on
the PE per phase and never hits HBM.

Threshold strategy (replaces the exact-rank machinery of the old kernel):
 - exact per-token sigma of flow values via the pattern Gram matrix
   (tiny [16,16] AllReduce, overlapped with the preamble),
 - Gaussian quantile seed t0 = z(kk/DD) * sigma,
 - P1: one fp32r flow pass storing |F|*inten as fp16 (128KB/partition),
 - two-stage count ladder (3+3 rungs) on the fp16 data with rungs placed at
   fp16-grid midpoints, so each rung count equals the exact fp32 count at the
   midpoint; log-log interpolation to count==kk.  Two tiny AllReduces.
 - P4: fp32 flow pass, mask |F*inten| >= th on f32, masked values cast fp16,
   fp16 2x dot-accumulate against xn*inten.
One AllGather of the per-core flow_out slices, then a replicated LN2 +
memory-MLP + FFN tail (fp32r matmuls).
"""

import os
from contextlib import ExitStack

import numpy as np

B, S, D, P = 1, 256, 512, 16
MAX_SEQ = 4096
NCORES = 8
ISLICE = D // NCORES          # 64 i-rows per core
FREE = ISLICE * D             # 32768 ij elements per token per core
NG = 2                        # token groups of 128
DD = D * D
NL1 = 2                       # stage-1 ladder rungs
NL2 = 3                       # stage-2 ladder rungs
DLT1 = float(os.environ.get("KERNEL_DLT1", "0.01"))
DLT2 = float(os.environ.get("KERNEL_DLT2", "0.0015"))
QW = FREE // 4                # ladder count quarter width (8192)

DEBUG = os.environ.get("KERNEL_DEBUG", "0") == "1"


def _host_constants():
    pos = np.arange(S, dtype=np.float64)
    inv = 1.0 / (10000.0 ** (np.arange(0, D, 2, dtype=np.float64) / D))
    ang = pos[:, None] * inv[None, :]
    sin = np.repeat(np.sin(ang), 2, axis=-1).astype(np.float32)
    cos = np.repeat(np.cos(ang), 2, axis=-1).astype(np.float32)
    # half-normal tail quantile z(q): P(|N(0,1)| >= z) = q, cubic in ln q
    qpoly = np.array([-0.0036756, -0.06789169, -0.73664117, 0.26370117], np.float32)
    return sin, cos, qpoly


def build_kernel():
    import concourse.bass as bass
    import concourse.mybir as mybir
    from concourse import bacc, masks
    from concourse.tile import TileContext

    dt = mybir.dt
    Alu = mybir.AluOpType
    Act = mybir.ActivationFunctionType
    AxX = mybir.AxisListType.X
    f32, f16 = dt.float32, dt.float16
    f32r = dt.float32r

    nc = bacc.Bacc("TRN2", num_devices=NCORES)

    bf16 = dt.bfloat16
    dp = nc.declare_dram_parameter
    x_in = dp("x", [S, D], f32, isOutput=False)
    pat_r = dp("pat_r", [P, FREE], f32r, isOutput=False)
    pat_hi = dp("pat_hi", [P, FREE], bf16, isOutput=False)
    pat_lo = dp("pat_lo", [P, FREE], bf16, isOutput=False)
    pat_T = dp("pat_T", [128, (FREE // 128) * P], f32, isOutput=False)
    sel_w1 = dp("sel_w1", [2 * D, 2 * P], f32, isOutput=False)
    sel_b1 = dp("sel_b1", [1, 2 * P], f32, isOutput=False)
    sel_w2 = dp("sel_w2", [2 * P, P], f32, isOutput=False)
    sel_b2 = dp("sel_b2", [1, P], f32, isOutput=False)
    win_w1 = dp("win_w1", [D, 64], f32, isOutput=False)
    win_b1 = dp("win_b1", [1, 64], f32, isOutput=False)
    win_w2 = dp("win_w2", [64, 1], f32, isOutput=False)
    win_b2 = dp("win_b2", [1, 1], f32, isOutput=False)
    int_w1 = dp("int_w1", [2 * D, 64], f32, isOutput=False)
    int_b1 = dp("int_b1", [1, 64], f32, isOutput=False)
    int_w2 = dp("int_w2", [64, 1], f32, isOutput=False)
    int_b2 = dp("int_b2", [1, 1], f32, isOutput=False)
    mem_w1 = dp("mem_w1", [2 * D, D], f32r, isOutput=False)
    mem_b1 = dp("mem_b1", [1, D], f32, isOutput=False)
    mem_w2 = dp("mem_w2", [D, D], f32r, isOutput=False)
    mem_b2 = dp("mem_b2", [1, D], f32, isOutput=False)
    memory_bank = dp("memory_bank", [512, D], f32, isOutput=False)
    up_w = dp("up_w", [D, 8 * D], f32r, isOutput=False)
    up_b = dp("up_b", [1, 8 * D], f32, isOutput=False)
    down_w = dp("down_w", [4 * D, D], f32r, isOutput=False)
    down_b = dp("down_b", [1, D], f32, isOutput=False)
    n1_g = dp("n1_g", [1, D], f32, isOutput=False)
    n1_b = dp("n1_b", [1, D], f32, isOutput=False)
    n2_g = dp("n2_g", [1, D], f32, isOutput=False)
    n2_b = dp("n2_b", [1, D], f32, isOutput=False)
    rope_sin = dp("rope_sin", [S, D], f32, isOutput=False)
    rope_cos = dp("rope_cos", [S, D], f32, isOutput=False)
    qpoly = dp("qpoly", [1, 4], f32, isOutput=False)
    out_dram = dp("out", [S, D], f32, isOutput=True)

    dbg = {}
    if DEBUG:
        for name, shape in [
            ("dbg_xn", [S, D]), ("dbg_xr", [S, D]), ("dbg_pw", [S, P]),
            ("dbg_inten", [S, 1]), ("dbg_scal", [1, 8]), ("dbg_t0", [S, 1]),
            ("dbg_cnt", [S, 3]), ("dbg_cnt2", [S, 3]), ("dbg_th", [S, 2]),
            ("dbg_fo", [S, D]), ("dbg_G", [P, P]), ("dbg_mid", [S, 3]),
        ]:
            dbg[name] = dp(name, shape, f32, isOutput=True)

    RG = [list(range(NCORES))]

    with ExitStack() as ctx:
        tc = ctx.enter_context(TileContext(nc))
        pw_ = ctx.enter_context(tc.tile_pool(name="persist", bufs=1))
        pool_mm = ctx.enter_context(tc.tile_pool(name="psumMM", bufs=3, space="PSUM"))
        pool_ps = ctx.enter_context(tc.tile_pool(name="psumT", bufs=2, space="PSUM"))
        pool_dram = ctx.enter_context(tc.tile_pool(name="dramst", bufs=1, space="DRAM"))

        def dma(dst, src):
            nc.sync.dma_start(out=dst, in_=src)

        def bcast_row(pool, src_dram_row, width, name, dtype=f32):
            t = pool.tile([128, width], dtype, name=name)
            dma(t[:], src_dram_row[:].to_broadcast([128, width]))
            return t

        identity = pw_.tile([128, 128], f32, name="identity")
        masks.make_identity(nc, identity[:])
        bc_n = [0]

        def pbcast(pool, dst_ap, src_ap, width, name):
            """broadcast [1,width] sbuf row to [128,width] via a DRAM bounce"""
            bc_n[0] += 1
            st = pool_dram.tile([1, width], f32, name=f"bc{bc_n[0]}_{name}")
            dma(st[:], src_ap)
            dma(dst_ap, st[:].to_broadcast([128, width]))

        def transpose_to(dst_ap, src_ap, name):
            p, f = src_ap.shape[0], src_ap.free_size()
            ps = pool_ps.tile([f, p], f32, name="Tps", tag="Tps",
                              padded_shape=[128, 128])
            nc.tensor.transpose(ps[:f, :p], src_ap, identity[:p, :p])
            nc.vector.tensor_copy(dst_ap, ps[:f, :p])  # rounds if dst is f32r

        def gelu_(pool, ap, name):
            e = pool.tile(list(ap.shape), f32, name=f"{name}_erf", tag="gelu_e")
            nc.scalar.activation(e[:], ap, Act.Erf, scale=float(1 / np.sqrt(2)))
            nc.vector.tensor_scalar(e[:], e[:], 1.0, 0.5, Alu.add, Alu.mult)
            nc.vector.tensor_tensor(ap, ap, e[:], Alu.mult)

        def silu_(pool, dst_ap, src_ap, name):
            sg = pool.tile(list(src_ap.shape), f32, name=f"{name}_sg", tag="silu_s")
            nc.scalar.activation(sg[:], src_ap, Act.Sigmoid)
            nc.vector.tensor_tensor(dst_ap, src_ap, sg[:], Alu.mult)

        # ---------- persistent tiles ----------
        xg = [pw_.tile([128, D], f32, name=f"xg{g}") for g in range(NG)]
        xn = [pw_.tile([128, D], f32, name=f"xn{g}") for g in range(NG)]
        pwt = [pw_.tile([P, 128], f32, name=f"pwT{g}") for g in range(NG)]
        pwt_r = [pw_.tile([P, 128], f32r, name=f"pwTr{g}") for g in range(NG)]
        pwt_hi = [pw_.tile([P, 128], bf16, name=f"pwTh{g}") for g in range(NG)]
        pwt_lo = [pw_.tile([P, 128], bf16, name=f"pwTl{g}") for g in range(NG)]
        pw_sb = [pw_.tile([128, P], f32, name=f"pwsb{g}") for g in range(NG)]
        inten = [pw_.tile([128, 1], f32, name=f"inten{g}") for g in range(NG)]
        kk_b = pw_.tile([128, 1], f32, name="kk_b")
        lkk_b = pw_.tile([128, 1], f32, name="lkk_b")
        zq_b = pw_.tile([128, 1], f32, name="zq_b")
        ones_sb = pw_.tile([128, 1], f32, name="ones_sb")
        nc.vector.memset(ones_sb[:], 1.0)
        t0 = [pw_.tile([128, 1], f32, name=f"t0_{g}") for g in range(NG)]
        th1 = [pw_.tile([128, 1], f32, name=f"th1_{g}") for g in range(NG)]
        th2 = [pw_.tile([128, 1], f32, name=f"th2_{g}") for g in range(NG)]
        G_sb = pw_.tile([P, P], f32, name="G_sb")

        for g in range(NG):
            dma(xg[g][:], x_in[g * 128:(g + 1) * 128, :])

        # =============== pattern Gram matrix (starts immediately;
        # AllReduce latency hides under the preamble) ===============
        G_stage = pool_dram.tile([P, P], f32, name="G_stage")
        G_out = pool_dram.tile([P, P], f32, name="G_out", addr_space="Shared")
        with tc.tile_pool(name="grampool", bufs=1) as gp0:
            G_ps = pool_ps.tile([P, P], f32, name="G_ps", tag="Tps",
                                padded_shape=[128, 128])
            NCHUNK = FREE // 128
            gTall = gp0.tile([128, NCHUNK * P], f32, name="gTall")
            dma(gTall[:], pat_T[:])
            for c in range(NCHUNK):
                nc.tensor.matmul(G_ps[:P, :P], gTall[:, c * P:(c + 1) * P],
                                 gTall[:, c * P:(c + 1) * P],
                                 start=(c == 0), stop=(c == NCHUNK - 1))
            G_loc = gp0.tile([P, P], f32, name="G_loc")
            nc.vector.tensor_copy(G_loc[:], G_ps[:P, :P])
            dma(G_stage[:], G_loc[:])
        nc.gpsimd.collective_compute(
            "AllReduce", Alu.add, replica_groups=RG,
            ins=[G_stage[:]], outs=[G_out[:]])
        dma(G_sb[:], G_out[:])
        if DEBUG:
            dma(dbg["dbg_G"][:], G_out[:])

        # =================== preamble (scoped pool) ===================
        with tc.tile_pool(name="preamble", bufs=1) as pp:
            sin_g, cos_g, xr = [], [], []
            for g in range(NG):
                t = pp.tile([128, D], f32, name=f"sin{g}")
                dma(t[:], rope_sin[g * 128:(g + 1) * 128, :])
                sin_g.append(t)
                t = pp.tile([128, D], f32, name=f"cos{g}")
                dma(t[:], rope_cos[g * 128:(g + 1) * 128, :])
                cos_g.append(t)
            n1g_b = bcast_row(pp, n1_g, D, "n1g_b")
            n1b_b = bcast_row(pp, n1_b, D, "n1b_b")

            for g in range(NG):
                mean = pp.tile([128, 1], f32, name=f"mean{g}")
                m2 = pp.tile([128, 1], f32, name=f"m2ln{g}")
                tmp = pp.tile([128, D], f32, name=f"lntmp{g}")
                nc.vector.tensor_reduce(mean[:], xg[g][:], AxX, Alu.add)
                nc.vector.tensor_scalar(mean[:], mean[:], 1.0 / D, None, Alu.mult)
                nc.vector.tensor_scalar(tmp[:], xg[g][:], mean[:], None, Alu.subtract)
                nc.vector.scalar_tensor_tensor(tmp[:], tmp[:], 1.0, tmp[:], Alu.mult,
                                               Alu.mult, accum_out=m2[:])
                nc.vector.tensor_scalar(m2[:], m2[:], 1.0 / D, 1e-5, Alu.mult, Alu.add)
                rstd = pp.tile([128, 1], f32, name=f"rstd{g}")
                nc.scalar.activation(rstd[:], m2[:], Act.Sqrt)
                nc.vector.reciprocal(rstd[:], rstd[:])
                nc.vector.tensor_scalar(xn[g][:], xg[g][:], mean[:], rstd[:],
                                        Alu.subtract, Alu.mult)
                nc.vector.scalar_tensor_tensor(xn[g][:], xn[g][:], 1.0, n1g_b[:],
                                               Alu.mult, Alu.mult)
                nc.vector.tensor_tensor(xn[g][:], xn[g][:], n1b_b[:], Alu.add)
                t_xr = pp.tile([128, D], f32, name=f"xr{g}")
                rot = pp.tile([128, D], f32, name=f"rot{g}")
                ev = lambda a: a.rearrange("p (a two) -> p a two", two=2)[:, :, 0]
                od = lambda a: a.rearrange("p (a two) -> p a two", two=2)[:, :, 1]
                nc.vector.tensor_scalar(ev(rot[:]), od(xn[g][:]), -1.0, None, Alu.mult)
                nc.vector.tensor_copy(od(rot[:]), ev(xn[g][:]))
                nc.vector.tensor_tensor(rot[:], rot[:], sin_g[g][:], Alu.mult)
                nc.vector.scalar_tensor_tensor(t_xr[:], xn[g][:], 1.0, cos_g[g][:],
                                               Alu.mult, Alu.mult)
                nc.vector.tensor_tensor(t_xr[:], t_xr[:], rot[:], Alu.add)
                xr.append(t_xr)

            # ctx = mean over tokens
            ctx_ps = pool_ps.tile([1, D], f32, name="ctx_ps", tag="Tps",
                                  padded_shape=[128, 512])
            for g in range(NG):
                nc.tensor.matmul(ctx_ps[:1, :], ones_sb[:], xr[g][:],
                                 start=(g == 0), stop=(g == NG - 1))
            ctx_row = pp.tile([1, D], f32, name="ctx_row")
            nc.vector.tensor_scalar(ctx_row[:], ctx_ps[:1, :], 1.0 / S, None, Alu.mult)

            xrT = pp.tile([128, 4 * S], f32, name="xrT")
            for g in range(NG):
                for kc in range(4):
                    transpose_to(xrT[:, kc * S + g * 128: kc * S + (g + 1) * 128],
                                 xr[g][:, kc * 128:(kc + 1) * 128], f"xrT{g}{kc}")
            ctxT = pp.tile([128, 4], f32, name="ctxT")
            for kc in range(4):
                transpose_to(ctxT[:, kc:kc + 1], ctx_row[:, kc * 128:(kc + 1) * 128],
                             f"ctxT{kc}")

            def mlp_head(w1, b1, w2, b2, h1_dim, h2_dim, name):
                w1a = pp.tile([128, 4 * h1_dim], f32, name=f"{name}_w1a")
                w1b = pp.tile([128, 4 * h1_dim], f32, name=f"{name}_w1b")
                for kc in range(4):
                    dma(w1a[:, kc * h1_dim:(kc + 1) * h1_dim],
                        w1[kc * 128:(kc + 1) * 128, :])
                    dma(w1b[:, kc * h1_dim:(kc + 1) * h1_dim],
                        w1[D + kc * 128: D + (kc + 1) * 128, :])
                b1_b = bcast_row(pp, b1, h1_dim, f"{name}_b1b")
                w2_sb = pp.tile([h1_dim, h2_dim], f32, name=f"{name}_w2sb")
                dma(w2_sb[:], w2[:])
                b2_b = bcast_row(pp, b2, h2_dim, f"{name}_b2b")
                v1_ps = pool_ps.tile([1, h1_dim], f32, name="v1ps", tag="Tps",
                                     padded_shape=[128, 128])
                for kc in range(4):
                    nc.tensor.matmul(v1_ps[:1, :], ctxT[:, kc:kc + 1],
                                     w1b[:, kc * h1_dim:(kc + 1) * h1_dim],
                                     start=(kc == 0), stop=(kc == 3))
                v1 = pp.tile([1, h1_dim], f32, name=f"{name}_v1")
                nc.vector.tensor_copy(v1[:], v1_ps[:1, :])
                v1_b = pp.tile([128, h1_dim], f32, name=f"{name}_v1b")
                pbcast(pp, v1_b[:], v1[:], h1_dim, f"{name}v1")
                outs = []
                for g in range(NG):
                    h1_ps = pool_ps.tile([128, h1_dim], f32, name="h1ps", tag="Tps",
                                         padded_shape=[128, 128])
                    for kc in range(4):
                        nc.tensor.matmul(
                            h1_ps[:], xrT[:, kc * S + g * 128: kc * S + (g + 1) * 128],
                            w1a[:, kc * h1_dim:(kc + 1) * h1_dim],
                            start=(kc == 0), stop=(kc == 3))
                    h1 = pp.tile([128, h1_dim], f32, name=f"{name}_h1_{g}")
                    nc.vector.tensor_tensor(h1[:], h1_ps[:], v1_b[:], Alu.add)
                    nc.vector.tensor_tensor(h1[:], h1[:], b1_b[:], Alu.add)
                    gelu_(pp, h1[:], f"{name}g{g}")
                    h1T = pp.tile([h1_dim, 128], f32, name=f"{name}_h1T_{g}")
                    transpose_to(h1T[:], h1[:], f"{name}h1T{g}")
                    h2_ps = pool_ps.tile([128, h2_dim], f32, name="h2ps", tag="Tps",
                                         padded_shape=[128, 128])
                    nc.tensor.matmul(h2_ps[:], h1T[:], w2_sb[:], start=True, stop=True)
                    h2 = pp.tile([128, h2_dim], f32, name=f"{name}_h2_{g}")
                    nc.vector.tensor_tensor(h2[:], h2_ps[:], b2_b[:], Alu.add)
                    outs.append(h2)
                return outs

            sel_h2 = mlp_head(sel_w1, sel_b1, sel_w2, sel_b2, 2 * P, P, "sel")
            int_h2 = mlp_head(int_w1, int_b1, int_w2, int_b2, 64, 1, "intm")

            for g in range(NG):
                mx = pp.tile([128, 1], f32, name=f"selmx{g}")
                nc.vector.tensor_reduce(mx[:], sel_h2[g][:], AxX, Alu.max)
                nc.vector.tensor_scalar(sel_h2[g][:], sel_h2[g][:], mx[:], None,
                                        Alu.subtract)
                nc.scalar.activation(sel_h2[g][:], sel_h2[g][:], Act.Exp)
                sm = pp.tile([128, 1], f32, name=f"selsm{g}")
                nc.vector.tensor_reduce(sm[:], sel_h2[g][:], AxX, Alu.add)
                rs = pp.tile([128, 1], f32, name=f"selrs{g}")
                nc.vector.reciprocal(rs[:], sm[:])
                nc.vector.tensor_scalar(pw_sb[g][:], sel_h2[g][:], rs[:], None,
                                        Alu.mult)
                nc.scalar.activation(inten[g][:], int_h2[g][:], Act.Sigmoid)
                transpose_to(pwt[g][:], pw_sb[g][:], f"pwT{g}")
                nc.vector.tensor_copy(pwt_r[g][:], pwt[g][:])
                # bf16 hi/lo split of pw for the precise P4 matmul
                nc.vector.tensor_copy(pwt_hi[g][:], pwt[g][:])
                hi32 = pp.tile([P, 128], f32, name=f"hi32_{g}")
                nc.vector.tensor_copy(hi32[:], pwt_hi[g][:])
                nc.vector.tensor_tensor(hi32[:], pwt[g][:], hi32[:], Alu.subtract)
                nc.vector.tensor_copy(pwt_lo[g][:], hi32[:])
                if DEBUG:
                    dma(dbg["dbg_pw"][g * 128:(g + 1) * 128, :], pw_sb[g][:])

            # window scalar -> kk, z
            winw1_sb = pp.tile([128, 4 * 64], f32, name="winw1_sb")
            for kc in range(4):
                dma(winw1_sb[:, kc * 64:(kc + 1) * 64],
                    win_w1[kc * 128:(kc + 1) * 128, :])
            wh1_ps = pool_ps.tile([1, 64], f32, name="wh1ps", tag="Tps",
                                  padded_shape=[128, 128])
            for kc in range(4):
                nc.tensor.matmul(wh1_ps[:1, :], ctxT[:, kc:kc + 1],
                                 winw1_sb[:, kc * 64:(kc + 1) * 64],
                                 start=(kc == 0), stop=(kc == 3))
            wh1 = pp.tile([1, 64], f32, name="wh1")
            wb1_sb = pp.tile([1, 64], f32, name="wb1_sb")
            dma(wb1_sb[:], win_b1[:])
            nc.vector.tensor_tensor(wh1[:], wh1_ps[:1, :], wb1_sb[:], Alu.add)
            gelu_(pp, wh1[:], "wh1g")
            wh1T = pp.tile([64, 1], f32, name="wh1T")
            transpose_to(wh1T[:], wh1[:], "wh1T")
            winw2_sb = pp.tile([64, 1], f32, name="winw2_sb")
            dma(winw2_sb[:], win_w2[:])
            win_ps = pool_ps.tile([1, 1], f32, name="winps", tag="Tps",
                                  padded_shape=[128, 128])
            nc.tensor.matmul(win_ps[:1, :1], wh1T[:], winw2_sb[:], start=True,
                             stop=True)
            winv = pp.tile([1, 1], f32, name="winv")
            wb2_sb = pp.tile([1, 1], f32, name="wb2_sb")
            dma(wb2_sb[:], win_b2[:])
            nc.vector.tensor_tensor(winv[:], win_ps[:1, :1], wb2_sb[:], Alu.add)
            nc.scalar.activation(winv[:], winv[:], Act.Sigmoid)
            nc.vector.tensor_scalar(winv[:], winv[:], float(MAX_SEQ - 256), 256.0,
                                    Alu.mult, Alu.add)
            kkf = pp.tile([1, 1], f32, name="kkf")
            nc.vector.tensor_scalar(kkf[:], winv[:], 0.1 / MAX_SEQ * DD, None,
                                    Alu.mult)
            # floor() robust to the f32->i32 convert rounding mode
            ki = pp.tile([1, 1], dt.int32, name="ki")
            nc.vector.tensor_copy(ki[:], kkf[:])
            kf2 = pp.tile([1, 1], f32, name="kf2")
            nc.vector.tensor_copy(kf2[:], ki[:])
            kgt = pp.tile([1, 1], f32, name="kgt")
            nc.vector.tensor_tensor(kgt[:], kf2[:], kkf[:], Alu.is_gt)
            nc.vector.tensor_tensor(kkf[:], kf2[:], kgt[:], Alu.subtract)
            nc.vector.tensor_scalar(kkf[:], kkf[:], 1.0, None, Alu.max)

            qp = pp.tile([1, 4], f32, name="qp")
            dma(qp[:], qpoly[:])
            u = pp.tile([1, 1], f32, name="qu")
            nc.vector.tensor_scalar(u[:], kkf[:], 1.0 / DD, None, Alu.mult)
            nc.scalar.activation(u[:], u[:], Act.Ln)
            zq = pp.tile([1, 1], f32, name="zq")
            nc.vector.tensor_scalar(zq[:], qp[:, 0:1], u[:], qp[:, 1:2], Alu.mult,
                                    Alu.add)
            nc.vector.tensor_scalar(zq[:], zq[:], u[:], qp[:, 2:3], Alu.mult, Alu.add)
            nc.vector.tensor_scalar(zq[:], zq[:], u[:], qp[:, 3:4], Alu.mult, Alu.add)
            pbcast(pp, kk_b[:], kkf[:], 1, "kk")
            pbcast(pp, zq_b[:], zq[:], 1, "zq")
            nc.scalar.activation(lkk_b[:], kk_b[:], Act.Ln)

            # sigma per token via Gram: q2 = pw^T G pw ; t0 = z*sqrt(q2/DD)*inten
            for g in range(NG):
                sig_ps = pool_ps.tile([128, P], f32, name="sigps", tag="Tps",
                                      padded_shape=[128, 128])
                nc.tensor.matmul(sig_ps[:], pwt[g][:], G_sb[:], start=True, stop=True)
                q2 = pp.tile([128, 1], f32, name=f"q2_{g}")
                scr = pp.tile([128, P], f32, name=f"q2scr{g}", tag="q2scr")
                nc.vector.scalar_tensor_tensor(scr[:], sig_ps[:], 1.0, pw_sb[g][:],
                                               Alu.mult, Alu.mult, accum_out=q2[:])
                sig = pp.tile([128, 1], f32, name=f"sig{g}")
                nc.scalar.activation(sig[:], q2[:], Act.Sqrt, scale=float(1.0 / DD))
                nc.vector.tensor_tensor(sig[:], sig[:], zq_b[:], Alu.mult)
                nc.vector.tensor_tensor(t0[g][:], sig[:], inten[g][:], Alu.mult)
                if DEBUG:
                    dma(dbg["dbg_t0"][g * 128:(g + 1) * 128, :], t0[g][:])

            if DEBUG:
                for g in range(NG):
                    dma(dbg["dbg_xn"][g * 128:(g + 1) * 128, :], xn[g][:])
                    dma(dbg["dbg_xr"][g * 128:(g + 1) * 128, :], xr[g][:])
                    dma(dbg["dbg_inten"][g * 128:(g + 1) * 128, :], inten[g][:])
                dma(dbg["dbg_scal"][:, 0:1], kkf[:])
                dma(dbg["dbg_scal"][:, 1:2], winv[:])
                dma(dbg["dbg_scal"][:, 2:3], zq[:])

        # =========== helpers: stream patterns & rematerialize F ===========
        # Both passes emit PAIRED chunks: one [128, 1024] 2-bank PSUM tile per
        # (c, c+1) so downstream scalar/vector ops amortize fixed costs.
        def flow_pass_r(g, consume, pat_pool):
            """fp32r pass (counting-grade precision)."""
            for w in range(16):
                patw = pat_pool.tile([P, 2048], f32r, name="patw", tag="patw",
                                     bufs=3)
                dma(patw[:], pat_r[:, w * 2048:(w + 1) * 2048])
                for m in range(0, 4, 2):
                    c = w * 4 + m
                    ps = pool_mm.tile([128, 1024], f32, name="Fps", tag="Fps")
                    for h in range(2):
                        nc.tensor.matmul(ps[:, h * 512:(h + 1) * 512],
                                         pwt_r[g][:],
                                         patw[:, (m + h) * 512:(m + h + 1) * 512],
                                         start=True, stop=True)
                    consume(c, ps)

        def flow_pass_hl(g, consume, pat_pool):
            """3-term bf16 split pass: hi*hi + hi*lo + lo*hi (~2^-16 precision,
            runs at full bf16 PE rate unlike fp32's half-rate 2-slice form)."""
            for w in range(16):
                pwh = pat_pool.tile([P, 2048], bf16, name="pwh", tag="pwh", bufs=3)
                pwl = pat_pool.tile([P, 2048], bf16, name="pwl", tag="pwl", bufs=3)
                dma(pwh[:], pat_hi[:, w * 2048:(w + 1) * 2048])
                dma(pwl[:], pat_lo[:, w * 2048:(w + 1) * 2048])
                for m in range(0, 4, 2):
                    c = w * 4 + m
                    ps = pool_mm.tile([128, 1024], f32, name="Fps", tag="Fps")
                    for h in range(2):
                        sl = slice((m + h) * 512, (m + h + 1) * 512)
                        po = ps[:, h * 512:(h + 1) * 512]
                        nc.tensor.matmul(po, pwt_hi[g][:], pwh[:, sl],
                                         start=True, stop=False)
                        nc.tensor.matmul(po, pwt_hi[g][:], pwl[:, sl],
                                         start=False, stop=False)
                        nc.tensor.matmul(po, pwt_lo[g][:], pwh[:, sl],
                                         start=False, stop=True)
                    consume(c, ps)

        # =============== ladder helpers ===============
        # g*(1-1.25*2^-11) lies 0.625..1.25 fp16-ULP below grid point g for any
        # mantissa, so RTN-to-fp16 lands exactly on the previous grid point.
        PREV16 = float(1.0 - 1.25 * 2.0 ** -11)

        def build_rungs(pool, center, scale_consts, g, name):
            """rungs at fp16-grid midpoints around center; returns (mids, lmids)"""
            nl = len(scale_consts)
            mids = pool.tile([128, nl], f32, name=f"{name}_mid{g}")
            lmids = pool.tile([128, nl], f32, name=f"{name}_lmid{g}")
            nmids = pool.tile([128, nl], f32, name=f"{name}_nmid{g}")
            graw = pool.tile([128, nl], f32, name=f"{name}_graw{g}")
            gf = pool.tile([128, nl], f32, name=f"{name}_gf{g}")
            g16 = pool.tile([128, nl], f16, name=f"{name}_g16{g}")
            gdec = pool.tile([128, nl], f16, name=f"{name}_gdec{g}")
            for j in range(nl):
                nc.vector.tensor_scalar(graw[:, j:j + 1], center[:],
                                        float(scale_consts[j]), None, Alu.mult)
            nc.vector.tensor_copy(g16[:], graw[:])              # rtn to fp16 grid
            nc.vector.tensor_copy(gf[:], g16[:])                # grid point, f32
            nc.vector.tensor_scalar(graw[:], gf[:], PREV16, None, Alu.mult)
            nc.vector.tensor_copy(gdec[:], graw[:])             # prev grid point
            nc.vector.tensor_copy(mids[:], gdec[:])
            nc.vector.tensor_tensor(mids[:], mids[:], gf[:], Alu.add)
            nc.vector.tensor_scalar(mids[:], mids[:], 0.5, None, Alu.mult)
            nc.vector.tensor_scalar(nmids[:], mids[:], -1.0, None, Alu.mult)
            nc.scalar.activation(lmids[:], mids[:], Act.Ln)
            return mids, lmids, nmids

        def count_rungs(pool, Ag, mids, nmids, nl, cl, scr_v, scr_s, g, name,
                        ns=2):
            """cl[:, j] = # (Ag >= mids[:, j]); quarters split scalar/vector
            (`ns` of 4 quarters on the scalar engine).

            Scalar quarters use Sign(A - mid) accumulated: S = #ge - #lt, so
            #ge = 0.5*S + QW/2 per quarter (mids sit strictly between fp16
            grid points, so A - mid never equals 0)."""
            ch = pool.tile([128, 5], f32, name=f"{name}_ch{g}", tag="cnt_ch")
            for j in range(nl):
                for q in range(4):
                    Aq = Ag[:, q * QW:(q + 1) * QW]
                    if q < ns:
                        nc.scalar.activation(scr_s[:], Aq, Act.Sign,
                                             bias=nmids[:, j:j + 1],
                                             accum_out=ch[:, q:q + 1])
                    else:
                        nc.vector.tensor_scalar(scr_v[:], Aq, mids[:, j:j + 1],
                                                None, Alu.is_ge, Alu.add,
                                                accum_out=ch[:, q:q + 1])
                if ns > 0:
                    nc.vector.tensor_reduce(cl[:, j:j + 1], ch[:, 0:ns], AxX,
                                            Alu.add)
                    nc.vector.tensor_scalar(cl[:, j:j + 1], cl[:, j:j + 1], 0.5,
                                            float(ns * QW // 2), Alu.mult, Alu.add)
                    nc.vector.tensor_reduce(ch[:, 4:5], ch[:, ns:4], AxX, Alu.add)
                    nc.vector.tensor_tensor(cl[:, j:j + 1], cl[:, j:j + 1],
                                            ch[:, 4:5], Alu.add)
                else:
                    nc.vector.tensor_reduce(cl[:, j:j + 1], ch[:, 0:4], AxX,
                                            Alu.add)

        def interp2_th(pool, cl, lmids, th_out, g, name):
            """log-log linear interp of count->kk over 2 rungs."""
            lc = pool.tile([128, 2], f32, name=f"{name}_lc{g}")
            nc.vector.tensor_scalar(lc[:], cl[:], 1.0, None, Alu.max)
            nc.scalar.activation(lc[:], lc[:], Act.Ln)
            num = pool.tile([128, 1], f32, name=f"{name}_num{g}")
            den = pool.tile([128, 1], f32, name=f"{name}_den{g}")
            dl = pool.tile([128, 1], f32, name=f"{name}_dl{g}")
            nc.vector.tensor_scalar(num[:], lc[:, 0:1], lkk_b[:], None,
                                    Alu.subtract)
            nc.vector.tensor_scalar(den[:], lc[:, 0:1], lc[:, 1:2], None,
                                    Alu.subtract)
            nc.vector.tensor_scalar(den[:], den[:], 1e-5, None, Alu.max)
            nc.vector.tensor_scalar(dl[:], lmids[:, 1:2], lmids[:, 0:1], None,
                                    Alu.subtract)
            nc.vector.reciprocal(den[:], den[:])
            nc.vector.tensor_tensor(num[:], num[:], den[:], Alu.mult)
            nc.vector.tensor_tensor(num[:], num[:], dl[:], Alu.mult)
            nc.vector.tensor_scalar(num[:], num[:], lmids[:, 0:1], None, Alu.add)
            nc.scalar.activation(th_out[:], num[:], Act.Exp)

        def interp_th(pool, cl, lmids, th_out, g, name):
            """log-log piecewise-linear interp of count->kk over 3 rungs."""
            lc = pool.tile([128, 3], f32, name=f"{name}_lc{g}")
            nc.vector.tensor_scalar(lc[:], cl[:], 1.0, None, Alu.max)
            nc.scalar.activation(lc[:], lc[:], Act.Ln)
            shi = pool.tile([128, 1], f32, name=f"{name}_shi{g}")
            nc.vector.tensor_scalar(shi[:], cl[:, 1:2], kk_b[:], None, Alu.is_ge)
            slo = pool.tile([128, 1], f32, name=f"{name}_slo{g}")
            nc.vector.tensor_scalar(slo[:], shi[:], -1.0, 1.0, Alu.mult, Alu.add)

            def blend(dst, a_hi, a_lo, tmp):
                nc.vector.tensor_tensor(dst, a_hi, shi[:], Alu.mult)
                nc.vector.tensor_tensor(tmp, a_lo, slo[:], Alu.mult)
                nc.vector.tensor_tensor(dst, dst, tmp, Alu.add)

            tmp = pool.tile([128, 1], f32, name=f"{name}_tmp{g}")
            num = pool.tile([128, 1], f32, name=f"{name}_num{g}")
            den = pool.tile([128, 1], f32, name=f"{name}_den{g}")
            base = pool.tile([128, 1], f32, name=f"{name}_base{g}")
            dl = pool.tile([128, 1], f32, name=f"{name}_dl{g}")
            d01 = pool.tile([128, 1], f32, name=f"{name}_d01{g}")
            d12 = pool.tile([128, 1], f32, name=f"{name}_d12{g}")
            # num = (lc[seg_lo_idx] - lkk)
            nc.vector.tensor_scalar(d01[:], lc[:, 1:2], lkk_b[:], None, Alu.subtract)
            nc.vector.tensor_scalar(d12[:], lc[:, 0:1], lkk_b[:], None, Alu.subtract)
            blend(num[:], d01[:], d12[:], tmp[:])
            # den = (lc[lo] - lc[hi])
            nc.vector.tensor_scalar(d01[:], lc[:, 1:2], lc[:, 2:3], None, Alu.subtract)
            nc.vector.tensor_scalar(d12[:], lc[:, 0:1], lc[:, 1:2], None, Alu.subtract)
            blend(den[:], d01[:], d12[:], tmp[:])
            nc.vector.tensor_scalar(den[:], den[:], 1e-5, None, Alu.max)
            # base / dl
            blend(base[:], lmids[:, 1:2], lmids[:, 0:1], tmp[:])
            nc.vector.tensor_scalar(d01[:], lmids[:, 2:3], lmids[:, 1:2], None,
                                    Alu.subtract)
            nc.vector.tensor_scalar(d12[:], lmids[:, 1:2], lmids[:, 0:1], None,
                                    Alu.subtract)
            blend(dl[:], d01[:], d12[:], tmp[:])
            nc.vector.reciprocal(den[:], den[:])
            nc.vector.tensor_tensor(num[:], num[:], den[:], Alu.mult)
            nc.vector.tensor_tensor(num[:], num[:], dl[:], Alu.mult)
            nc.vector.tensor_tensor(base[:], base[:], num[:], Alu.add)
            nc.scalar.activation(th_out[:], base[:], Act.Exp)

        # =============== P1: |F| -> fp16 + two-stage ladder ===============
        t_stage = pool_dram.tile([S, NL1], f32, name="t_stage")
        t_out = pool_dram.tile([S, NL1], f32, name="t_out", addr_space="Shared")
        t2_stage = pool_dram.tile([S, NL2], f32, name="t2_stage")
        t2_out = pool_dram.tile([S, NL2], f32, name="t2_out", addr_space="Shared")

        e1 = [float(np.exp(-DLT1)), float(np.exp(DLT1))]
        e2 = [float(np.exp(-DLT2)), 1.0, float(np.exp(DLT2))]

        with tc.tile_pool(name="selpool", bufs=1) as sp:
            A16 = sp.tile([128, NG * FREE], f16, name="A16")
            scr_v = sp.tile([128, QW], f16, name="scr_v")
            scr_s = sp.tile([128, QW], f16, name="scr_s")

            for g in range(NG):
                def consume_p1(c, ps, g=g):
                    nc.scalar.activation(
                        A16[:, g * FREE + c * 512: g * FREE + (c + 2) * 512],
                        ps[:], Act.Abs, scale=inten[g][:])
                flow_pass_r(g, consume_p1, sp)

            # stage 1
            lm1 = []
            for g in range(NG):
                mids, lmids, nmids = build_rungs(sp, t0[g], e1, g, "s1")
                lm1.append(lmids)
                cl = sp.tile([128, NL1], f32, name=f"cl1_{g}")
                count_rungs(sp, A16[:, g * FREE:(g + 1) * FREE], mids, nmids,
                            NL1, cl, scr_v, scr_s, g, "s1", ns=1)
                dma(t_stage[g * 128:(g + 1) * 128, :], cl[:])
                if DEBUG:
                    dma(dbg["dbg_mid"][g * 128:(g + 1) * 128, 0:NL1], mids[:])
            nc.gpsimd.collective_compute(
                "AllReduce", Alu.add, replica_groups=RG,
                ins=[t_stage[:]], outs=[t_out[:]])
            for g in range(NG):
                cl = sp.tile([128, NL1], f32, name=f"cl1g_{g}")
                dma(cl[:], t_out[g * 128:(g + 1) * 128, :])
                if DEBUG:
                    dma(dbg["dbg_cnt"][g * 128:(g + 1) * 128, 0:NL1], cl[:])
                interp2_th(sp, cl, lm1[g], th1[g][:], g, "i1")

            # stage 2
            lm2 = []
            for g in range(NG):
                mids, lmids, nmids = build_rungs(sp, th1[g], e2, g, "s2")
                lm2.append(lmids)
                cl = sp.tile([128, NL2], f32, name=f"cl2_{g}")
                count_rungs(sp, A16[:, g * FREE:(g + 1) * FREE], mids, nmids,
                            NL2, cl, scr_v, scr_s, g, "s2", ns=2 + g)
                dma(t2_stage[g * 128:(g + 1) * 128, :], cl[:])
            nc.gpsimd.collective_compute(
                "AllReduce", Alu.add, replica_groups=RG,
                ins=[t2_stage[:]], outs=[t2_out[:]])
            for g in range(NG):
                cl = sp.tile([128, NL2], f32, name=f"cl2g_{g}")
                dma(cl[:], t2_out[g * 128:(g + 1) * 128, :])
                if DEBUG:
                    dma(dbg["dbg_cnt2"][g * 128:(g + 1) * 128, 0:NL2], cl[:])
                interp_th(sp, cl, lm2[g], th2[g][:], g, "i2")
                if DEBUG:
                    dma(dbg["dbg_th"][g * 128:(g + 1) * 128, 0:1], th1[g][:])
                    dma(dbg["dbg_th"][g * 128:(g + 1) * 128, 1:2], th2[g][:])

        # =============== P4: final masked matvec (fp32 pass) ===============
        fo_stage = pool_dram.tile([S, ISLICE], f32, name="fo_stage")
        fo_out = pool_dram.tile([NCORES, S, ISLICE], f32, name="fo_out",
                                addr_space="Shared")
        tailP = ctx.enter_context(tc.tile_pool(name="tailP", bufs=1))
        fo_full = [tailP.tile([128, D], f32, name=f"fo_full{g}") for g in range(NG)]
        with tc.tile_pool(name="p4pool", bufs=1) as fp:
            XI16 = []
            for g in range(NG):
                t = fp.tile([128, D], f16, name=f"XI16_{g}")
                nc.vector.tensor_scalar(t[:], xn[g][:], inten[g][:], None, Alu.mult)
                XI16.append(t)
            for g in range(NG):
                FO = fp.tile([128, ISLICE], f32, name=f"FO{g}")

                def consume_p4(c, ps, g=g, FO=FO):
                    At = fp.tile([128, 1024], f32, name="At", tag="At", bufs=3)
                    FM = fp.tile([128, 1024], f16, name="FM", tag="FM", bufs=3)
                    sc16 = fp.tile([128, 1024], f16, name="sc16", tag="sc16",
                                   bufs=3)
                    nc.scalar.activation(At[:], ps[:], Act.Abs, scale=inten[g][:])
                    nc.vector.scalar_tensor_tensor(FM[:], At[:], th2[g][:], ps[:],
                                                   Alu.is_ge, Alu.mult)
                    for h in range(2):
                        nc.vector.scalar_tensor_tensor(
                            sc16[:, h * 512:(h + 1) * 512],
                            FM[:, h * 512:(h + 1) * 512], 1.0, XI16[g][:],
                            Alu.mult, Alu.mult,
                            accum_out=FO[:, c + h:c + h + 1])
                flow_pass_hl(g, consume_p4, fp)
                dma(fo_stage[g * 128:(g + 1) * 128, :], FO[:])

        nc.gpsimd.collective_compute(
            "AllGather", Alu.bypass, replica_groups=RG,
            ins=[fo_stage[:]], outs=[fo_out[:]])

        # =============== tail ===============
        co = [tailP.tile([128, D], f32, name=f"co{g}") for g in range(NG)]
        with tc.tile_pool(name="tail1", bufs=1) as tp:
            n2g_b = bcast_row(tp, n2_g, D, "n2g_b")
            n2b_b = bcast_row(tp, n2_b, D, "n2b_b")
            for g in range(NG):
                for cidx in range(NCORES):
                    dma(fo_full[g][:, cidx * ISLICE:(cidx + 1) * ISLICE],
                        fo_out[cidx, g * 128:(g + 1) * 128, :])
                if DEBUG:
                    dma(dbg["dbg_fo"][g * 128:(g + 1) * 128, :], fo_full[g][:])
                nc.vector.tensor_tensor(co[g][:], xg[g][:], fo_full[g][:], Alu.add)
                mean = tp.tile([128, 1], f32, name=f"mean2{g}")
                m2 = tp.tile([128, 1], f32, name=f"m2ln2{g}")
                tmp = tp.tile([128, D], f32, name=f"ln2tmp{g}", tag="tmp")
                nc.vector.tensor_reduce(mean[:], co[g][:], AxX, Alu.add)
                nc.vector.tensor_scalar(mean[:], mean[:], 1.0 / D, None, Alu.mult)
                nc.vector.tensor_scalar(tmp[:], co[g][:], mean[:], None,
                                        Alu.subtract)
                nc.vector.scalar_tensor_tensor(tmp[:], tmp[:], 1.0, tmp[:], Alu.mult,
                                               Alu.mult, accum_out=m2[:])
                nc.vector.tensor_scalar(m2[:], m2[:], 1.0 / D, 1e-5, Alu.mult,
                                        Alu.add)
                rstd = tp.tile([128, 1], f32, name=f"rstd2{g}")
                nc.scalar.activation(rstd[:], m2[:], Act.Sqrt)
                nc.vector.reciprocal(rstd[:], rstd[:])
                nc.vector.tensor_scalar(co[g][:], co[g][:], mean[:], rstd[:],
                                        Alu.subtract, Alu.mult)
                nc.vector.scalar_tensor_tensor(co[g][:], co[g][:], 1.0, n2g_b[:],
                                               Alu.mult, Alu.mult)
                nc.vector.tensor_tensor(co[g][:], co[g][:], n2b_b[:], Alu.add)

        def transposed_cols(pool, src_list, K, name):
            nk = K // 128
            tT = pool.tile([128, nk * S], f32r, name=f"{name}_T")
            for g in range(NG):
                for kc in range(nk):
                    transpose_to(tT[:, kc * S + g * 128: kc * S + (g + 1) * 128],
                                 src_list[g][:, kc * 128:(kc + 1) * 128],
                                 f"{name}T{g}_{kc}")
            return lambda g, kc: tT[:, kc * S + g * 128: kc * S + (g + 1) * 128]

        def big_matmul(pool, lhsT_cols, w_dram, K, N, name, bias_dram=None,
                       const_lhsT=None, out_list=None):
            nk = K // 128
            wsb = pool.tile([128, nk * N], f32r, name=f"{name}_wsb")
            for kc in range(nk):
                dma(wsb[:, kc * N:(kc + 1) * N], w_dram[kc * 128:(kc + 1) * 128, :])
            bias_b = (bcast_row(pool, bias_dram, N, f"{name}_bias")
                      if bias_dram is not None else None)
            cvec_b = None
            if const_lhsT is not None:
                cps = pool_ps.tile([1, N], f32, name="cps", tag="Tps",
                                   padded_shape=[128, 512])
                for kc in range(nk):
                    nc.tensor.matmul(cps[:1, :], const_lhsT[:, kc:kc + 1],
                                     wsb[:, kc * N:(kc + 1) * N],
                                     start=(kc == 0), stop=(kc == nk - 1))
                cvec = pool.tile([1, N], f32, name=f"{name}_cvec")
                nc.vector.tensor_copy(cvec[:], cps[:1, :])
                cvec_b = pool.tile([128, N], f32, name=f"{name}_cvecb")
                pbcast(pool, cvec_b[:], cvec[:], N, f"{name}cv")
            outs = []
            for g in range(NG):
                o = (out_list[g] if out_list is not None
                     else pool.tile([128, N], f32, name=f"{name}_o{g}"))
                for nb in range(0, N, 1024):
                    nw = min(1024, N - nb)
                    ps = pool_mm.tile([128, 1024], f32, name="Fps", tag="Fps")
                    for h in range(0, nw, 512):
                        hw = min(512, nw - h)
                        for kc in range(nk):
                            nc.tensor.matmul(
                                ps[:, h:h + hw], lhsT_cols(g, kc),
                                wsb[:, kc * N + nb + h: kc * N + nb + h + hw],
                                start=(kc == 0), stop=(kc == nk - 1))
                    nc.vector.tensor_copy(o[:, nb:nb + nw], ps[:, :nw])
                if bias_b is not None:
                    nc.vector.tensor_tensor(o[:], o[:], bias_b[:], Alu.add)
                if cvec_b is not None:
                    nc.vector.tensor_tensor(o[:], o[:], cvec_b[:], Alu.add)
                outs.append(o)
            return outs

        # memory-bank mean -> memvT [D,1] as 4 chunks
        with tc.tile_pool(name="tailmem", bufs=1) as mp:
            memx = mp.tile([128, 4 * D], f32, name="memx")
            for kc in range(4):
                dma(memx[:, kc * D:(kc + 1) * D],
                    memory_bank[kc * 128:(kc + 1) * 128, :])
            mem_ps = pool_ps.tile([1, D], f32, name="memps", tag="Tps",
                                  padded_shape=[128, 512])
            for kc in range(4):
                nc.tensor.matmul(mem_ps[:1, :], ones_sb[:],
                                 memx[:, kc * D:(kc + 1) * D],
                                 start=(kc == 0), stop=(kc == 3))
            memv = mp.tile([1, D], f32, name="memv")
            nc.vector.tensor_scalar(memv[:], mem_ps[:1, :], 1.0 / 512.0, None,
                                    Alu.mult)
            memvT = tailP.tile([128, 4], f32r, name="memvT")
            for kc in range(4):
                transpose_to(memvT[:, kc:kc + 1], memv[:, kc * 128:(kc + 1) * 128],
                             f"memvT{kc}")

        with tc.tile_pool(name="tailA", bufs=1) as ta_:
            coT = transposed_cols(ta_, co, D, "coT")
            mh = big_matmul(ta_, coT, mem_w1, D, D, "memh", bias_dram=mem_b1,
                            const_lhsT=memvT)
            for g in range(NG):
                silu_(ta_, mh[g][:], mh[g][:], f"mh{g}")
            mhT = transposed_cols(ta_, mh, D, "mhT")
            mo = big_matmul(ta_, mhT, mem_w2, D, D, "memo", bias_dram=mem_b2)
            for g in range(NG):
                nc.vector.tensor_tensor(co[g][:], co[g][:], mo[g][:], Alu.add)

        gv = [tailP.tile([128, 4 * D], f32, name=f"gv{g}") for g in range(NG)]
        with tc.tile_pool(name="tailB", bufs=1) as tb_:
            coT2 = transposed_cols(tb_, co, D, "coT2")
            ff = big_matmul(tb_, coT2, up_w, D, 8 * D, "ff", bias_dram=up_b)
            for g in range(NG):
                silu_(tb_, gv[g][:], ff[g][:, :4 * D], f"gv{g}")
                nc.vector.tensor_tensor(gv[g][:], gv[g][:], ff[g][:, 4 * D:],
                                        Alu.mult)
        with tc.tile_pool(name="tailC", bufs=1) as tcp:
            gvT = transposed_cols(tcp, gv, 4 * D, "gvT")
            ffn = big_matmul(tcp, gvT, down_w, 4 * D, D, "ffn", bias_dram=down_b)
            for g in range(NG):
                nc.vector.tensor_tensor(ffn[g][:], ffn[g][:], co[g][:], Alu.add)
                dma(out_dram[g * 128:(g + 1) * 128, :], ffn[g][:])

    return nc


def _install_ntff_shim():
    """Reconstitute the missing antenv.axon_hooks module so
    run_bass_kernel_spmd(trace=True) can reach the axon NTFF profiler."""
    import sys
    import types

    if "antenv.axon_hooks" in sys.modules:
        return
    import antenv

    mod = types.ModuleType("antenv.axon_hooks")
    _h = [None]
    mod.set_axon_ntff_profile_hook = lambda h: _h.__setitem__(0, h)
    mod.get_axon_ntff_profile_hook = lambda: _h[0]
    sys.modules["antenv.axon_hooks"] = mod
    antenv.axon_hooks = mod
    try:
        from trn_agent_boot.trn_boot import _ntff_profile_via_ctypes

        mod.set_axon_ntff_profile_hook(
            _ntff_profile_via_ctypes("/opt/axon/libaxon_pjrt.so"))
    except Exception:
        pass


def kernel(**inputs):
    from concourse.bass_utils import run_bass_kernel_spmd
    _install_ntff_shim()

    sin, cos, qpoly = _host_constants()
    x = np.ascontiguousarray(np.asarray(inputs["x"], np.float32).reshape(S, D))
    patterns = np.ascontiguousarray(np.asarray(inputs["flow_patterns"], np.float32))

    nc = build_kernel()
    nc.finalize()

    def a(k):
        return np.ascontiguousarray(np.asarray(inputs[k], np.float32))

    def row(k):
        return np.ascontiguousarray(np.asarray(inputs[k], np.float32).reshape(1, -1))

    base = {
        "x": x,
        "sel_w1": a("sel_w1"), "sel_b1": row("sel_b1"),
        "sel_w2": a("sel_w2"), "sel_b2": row("sel_b2"),
        "win_w1": a("win_w1"), "win_b1": row("win_b1"),
        "win_w2": a("win_w2"), "win_b2": row("win_b2"),
        "int_w1": a("int_w1"), "int_b1": row("int_b1"),
        "int_w2": a("int_w2"), "int_b2": row("int_b2"),
        "mem_w1": a("mem_w1"), "mem_b1": row("mem_b1"),
        "mem_w2": a("mem_w2"), "mem_b2": row("mem_b2"),
        "memory_bank": a("memory_bank"),
        "up_w": a("up_w"), "up_b": row("up_b"),
        "down_w": a("down_w"), "down_b": row("down_b"),
        "n1_g": row("n1_g"), "n1_b": row("n1_b"),
        "n2_g": row("n2_g"), "n2_b": row("n2_b"),
        "rope_sin": sin, "rope_cos": cos,
        "qpoly": qpoly.reshape(1, 4),
    }
    import ml_dtypes
    in_maps = []
    for c in range(NCORES):
        m = dict(base)
        psl = np.ascontiguousarray(
            patterns[:, c * ISLICE:(c + 1) * ISLICE, :].reshape(P, FREE))
        m["pat_r"] = psl
        phi = psl.astype(ml_dtypes.bfloat16)
        m["pat_hi"] = phi
        m["pat_lo"] = (psl - phi.astype(np.float32)).astype(ml_dtypes.bfloat16)
        # [FREE, P] -> [128, (FREE/128)*P]: partition p holds rows p, p+128, ...
        m["pat_T"] = np.ascontiguousarray(
            psl.T.reshape(FREE // 128, 128, P).transpose(1, 0, 2).reshape(
                128, (FREE // 128) * P))
        in_maps.append(m)

    trace = os.environ.get("KERNEL_TRACE", "0") == "1"
    res = run_bass_kernel_spmd(nc, in_maps, list(range(NCORES)), trace=trace)
    out0 = res.results[0]
    kernel.last_results = res.results
    kernel.last_exec_ns = getattr(res, "exec_time_ns", None)
    return out0["out"].reshape(B, S, D).astype(np.float32)


if __name__ == "__main__":
    data = np.load("/tmp/inputs.npz")
    inputs = {k: data[k] for k in data.files}
    out = kernel(**inputs)
    print("out", out.shape, float(np.abs(out).max()))


# revision 54
# speedup vs baseline: 1.0733x; 1.0733x over previous
"""Trainium2 Bass kernel for nn_EnhancedFlowLayer (topk_masking).

8 cores. Tokens on partitions (2 groups of 128); flow (i,j)-space sharded by i
across cores (64 i-rows -> 32768 elems/token/core). flow is rematerialized on
the PE per phase and never hits HBM.

Threshold strategy (replaces the exact-rank machinery of the old kernel):
 - exact per-token sigma of flow values via the pattern Gram matrix
   (tiny [16,16] AllReduce, overlapped with the preamble),
 - Gaussian quantile seed t0 = z(kk/DD) * sigma,
 - P1: one fp32r flow pass storing |F|*inten as fp16 (128KB/partition),
 - two-stage count ladder (3+3 rungs) on the fp16 data with rungs placed at
   fp16-grid midpoints, so each rung count equals the exact fp32 count at the
   midpoint; log-log interpolation to count==kk.  Two tiny AllReduces.
 - P4: fp32 flow pass, mask |F*inten| >= th on f32, masked values cast fp16,
   fp16 2x dot-accumulate against xn*inten.
One AllGather of the per-core flow_out slices, then a replicated LN2 +
memory-MLP + FFN tail (fp32r matmuls).
"""

import os
from contextlib import ExitStack

import numpy as np

B, S, D, P = 1, 256, 512, 16
MAX_SEQ = 4096
NCORES = 8
ISLICE = D // NCORES          # 64 i-rows per core
FREE = ISLICE * D             # 32768 ij elements per token per core
NG = 2                        # token groups of 128
DD = D * D
NL1 = 2                       # stage-1 ladder rungs
NL2 = 3                       # stage-2 ladder rungs
DLT1 = float(os.environ.get("KERNEL_DLT1", "0.01"))
DLT2 = float(os.environ.get("KERNEL_DLT2", "0.0015"))
QW = FREE // 4                # ladder count quarter width (8192)

DEBUG = os.environ.get("KERNEL_DEBUG", "0") == "1"


def _host_constants():
    pos = np.arange(S, dtype=np.float64)
    inv = 1.0 / (10000.0 ** (np.arange(0, D, 2, dtype=np.float64) / D))
    ang = pos[:, None] * inv[None, :]
    sin = np.repeat(np.sin(ang), 2, axis=-1).astype(np.float32)
    cos = np.repeat(np.cos(ang), 2, axis=-1).astype(np.float32)
    # half-normal tail quantile z(q): P(|N(0,1)| >= z) = q, cubic in ln q
    qpoly = np.array([-0.0036756, -0.06789169, -0.73664117, 0.26370117], np.float32)
    return sin, cos, qpoly


def build_kernel():
    import concourse.bass as bass
    import concourse.mybir as mybir
    from concourse import bacc, masks
    from concourse.tile import TileContext

    dt = mybir.dt
    Alu = mybir.AluOpType
    Act = mybir.ActivationFunctionType
    AxX = mybir.AxisListType.X
    f32, f16 = dt.float32, dt.float16
    f32r = dt.float32r

    nc = bacc.Bacc("TRN2", num_devices=NCORES)

    bf16 = dt.bfloat16
    dp = nc.declare_dram_parameter
    x_in = dp("x", [S, D], f32, isOutput=False)
    pat_r = dp("pat_r", [P, FREE], f32r, isOutput=False)
    pat_hi = dp("pat_hi", [P, FREE], bf16, isOutput=False)
    pat_lo = dp("pat_lo", [P, FREE], bf16, isOutput=False)
    pat_T = dp("pat_T", [128, (FREE // 128) * P], f32, isOutput=False)
    sel_w1 = dp("sel_w1", [2 * D, 2 * P], f32, isOutput=False)
    sel_b1 = dp("sel_b1", [1, 2 * P], f32, isOutput=False)
    sel_w2 = dp("sel_w2", [2 * P, P], f32, isOutput=False)
    sel_b2 = dp("sel_b2", [1, P], f32, isOutput=False)
    win_w1 = dp("win_w1", [D, 64], f32, isOutput=False)
    win_b1 = dp("win_b1", [1, 64], f32, isOutput=False)
    win_w2 = dp("win_w2", [64, 1], f32, isOutput=False)
    win_b2 = dp("win_b2", [1, 1], f32, isOutput=False)
    int_w1 = dp("int_w1", [2 * D, 64], f32, isOutput=False)
    int_b1 = dp("int_b1", [1, 64], f32, isOutput=False)
    int_w2 = dp("int_w2", [64, 1], f32, isOutput=False)
    int_b2 = dp("int_b2", [1, 1], f32, isOutput=False)
    f16d = dt.float16
    mem_w1 = dp("mem_w1", [2 * D, D], f16d, isOutput=False)
    mem_b1 = dp("mem_b1", [1, D], f32, isOutput=False)
    mem_w2 = dp("mem_w2", [D, D], f16d, isOutput=False)
    mem_b2 = dp("mem_b2", [1, D], f32, isOutput=False)
    mem_b2T = dp("mem_b2T", [128, 4], f32, isOutput=False)
    memory_bank = dp("memory_bank", [512, D], f32, isOutput=False)
    up_w = dp("up_w", [D, 8 * D], f16d, isOutput=False)
    up_b = dp("up_b", [1, 8 * D], f32, isOutput=False)
    up_bT = dp("up_bT", [128, 32], f32, isOutput=False)
    down_w = dp("down_w", [4 * D, D], f16d, isOutput=False)
    down_b = dp("down_b", [1, D], f32, isOutput=False)
    n1_g = dp("n1_g", [1, D], f32, isOutput=False)
    n1_b = dp("n1_b", [1, D], f32, isOutput=False)
    n2_g = dp("n2_g", [1, D], f32, isOutput=False)
    n2_b = dp("n2_b", [1, D], f32, isOutput=False)
    rope_sin = dp("rope_sin", [S, D], f32, isOutput=False)
    rope_cos = dp("rope_cos", [S, D], f32, isOutput=False)
    qpoly = dp("qpoly", [1, 4], f32, isOutput=False)
    out_dram = dp("out", [S, D], f32, isOutput=True)

    dbg = {}
    if DEBUG:
        for name, shape in [
            ("dbg_xn", [S, D]), ("dbg_xr", [S, D]), ("dbg_pw", [S, P]),
            ("dbg_inten", [S, 1]), ("dbg_scal", [1, 8]), ("dbg_t0", [S, 1]),
            ("dbg_cnt", [S, 3]), ("dbg_cnt2", [S, 3]), ("dbg_th", [S, 2]),
            ("dbg_fo", [S, D]), ("dbg_G", [P, P]), ("dbg_mid", [S, 3]),
        ]:
            dbg[name] = dp(name, shape, f32, isOutput=True)

    RG = [list(range(NCORES))]

    with ExitStack() as ctx:
        tc = ctx.enter_context(TileContext(nc))
        pw_ = ctx.enter_context(tc.tile_pool(name="persist", bufs=1))
        pool_ps = ctx.enter_context(tc.tile_pool(name="psumT", bufs=2, space="PSUM"))
        pool_dram = ctx.enter_context(tc.tile_pool(name="dramst", bufs=1, space="DRAM"))
        # flow-phase PSUM pool; manually closed after P4 so the tail can open
        # its own deeper PSUM pool within the 8-bank budget
        _flowmm_cm = tc.tile_pool(name="psumMM", bufs=3, space="PSUM")
        pool_mm = _flowmm_cm.__enter__()

        def dma(dst, src):
            nc.sync.dma_start(out=dst, in_=src)

        def bcast_row(pool, src_dram_row, width, name, dtype=f32):
            t = pool.tile([128, width], dtype, name=name)
            dma(t[:], src_dram_row[:].to_broadcast([128, width]))
            return t

        identity = pw_.tile([128, 128], f32, name="identity")
        masks.make_identity(nc, identity[:])
        bc_n = [0]

        def pbcast(pool, dst_ap, src_ap, width, name):
            """broadcast [1,width] sbuf row to [128,width] via a DRAM bounce"""
            bc_n[0] += 1
            st = pool_dram.tile([1, width], f32, name=f"bc{bc_n[0]}_{name}")
            dma(st[:], src_ap)
            dma(dst_ap, st[:].to_broadcast([128, width]))

        def transpose_to(dst_ap, src_ap, name):
            p, f = src_ap.shape[0], src_ap.free_size()
            ps = pool_ps.tile([f, p], f32, name="Tps", tag="Tps",
                              padded_shape=[128, 128])
            nc.tensor.transpose(ps[:f, :p], src_ap, identity[:p, :p])
            nc.vector.tensor_copy(dst_ap, ps[:f, :p])  # rounds if dst is f32r

        def gelu_(pool, ap, name):
            e = pool.tile(list(ap.shape), f32, name=f"{name}_erf", tag="gelu_e")
            nc.scalar.activation(e[:], ap, Act.Erf, scale=float(1 / np.sqrt(2)))
            nc.vector.tensor_scalar(e[:], e[:], 1.0, 0.5, Alu.add, Alu.mult)
            nc.vector.tensor_tensor(ap, ap, e[:], Alu.mult)

        def silu_(pool, dst_ap, src_ap, name):
            sg = pool.tile(list(src_ap.shape), f32, name=f"{name}_sg", tag="silu_s")
            nc.scalar.activation(sg[:], src_ap, Act.Sigmoid)
            nc.vector.tensor_tensor(dst_ap, src_ap, sg[:], Alu.mult)

        # ---------- persistent tiles ----------
        xg = [pw_.tile([128, D], f32, name=f"xg{g}") for g in range(NG)]
        xn = [pw_.tile([128, D], f32, name=f"xn{g}") for g in range(NG)]
        pwt = [pw_.tile([P, 128], f32, name=f"pwT{g}") for g in range(NG)]
        pwt_r = [pw_.tile([P, 128], f32r, name=f"pwTr{g}") for g in range(NG)]
        pwt_hi = [pw_.tile([P, 128], bf16, name=f"pwTh{g}") for g in range(NG)]
        pwt_lo = [pw_.tile([P, 128], bf16, name=f"pwTl{g}") for g in range(NG)]
        pw_sb = [pw_.tile([128, P], f32, name=f"pwsb{g}") for g in range(NG)]
        inten = [pw_.tile([128, 1], f32, name=f"inten{g}") for g in range(NG)]
        kk_b = pw_.tile([128, 1], f32, name="kk_b")
        lkk_b = pw_.tile([128, 1], f32, name="lkk_b")
        zq_b = pw_.tile([128, 1], f32, name="zq_b")
        ones_sb = pw_.tile([128, 1], f32, name="ones_sb")
        nc.vector.memset(ones_sb[:], 1.0)
        t0 = [pw_.tile([128, 1], f32, name=f"t0_{g}") for g in range(NG)]
        th1 = [pw_.tile([128, 1], f32, name=f"th1_{g}") for g in range(NG)]
        th2 = [pw_.tile([128, 1], f32, name=f"th2_{g}") for g in range(NG)]
        G_sb = pw_.tile([P, P], f32, name="G_sb")

        for g in range(NG):
            dma(xg[g][:], x_in[g * 128:(g + 1) * 128, :])

        # =============== pattern Gram matrix (starts immediately;
        # AllReduce latency hides under the preamble) ===============
        G_stage = pool_dram.tile([P, P], f32, name="G_stage")
        G_out = pool_dram.tile([P, P], f32, name="G_out", addr_space="Shared")
        with tc.tile_pool(name="grampool", bufs=1) as gp0:
            G_ps = pool_ps.tile([P, P], f32, name="G_ps", tag="Tps",
                                padded_shape=[128, 128])
            NCHUNK = FREE // 128
            gTall = gp0.tile([128, NCHUNK * P], f32, name="gTall")
            dma(gTall[:], pat_T[:])
            for c in range(NCHUNK):
                nc.tensor.matmul(G_ps[:P, :P], gTall[:, c * P:(c + 1) * P],
                                 gTall[:, c * P:(c + 1) * P],
                                 start=(c == 0), stop=(c == NCHUNK - 1))
            G_loc = gp0.tile([P, P], f32, name="G_loc")
            nc.vector.tensor_copy(G_loc[:], G_ps[:P, :P])
            dma(G_stage[:], G_loc[:])
        nc.gpsimd.collective_compute(
            "AllReduce", Alu.add, replica_groups=RG,
            ins=[G_stage[:]], outs=[G_out[:]])
        dma(G_sb[:], G_out[:])
        if DEBUG:
            dma(dbg["dbg_G"][:], G_out[:])

        # =================== preamble (scoped pool) ===================
        with tc.tile_pool(name="preamble", bufs=1) as pp:
            sin_g, cos_g, xr = [], [], []
            for g in range(NG):
                t = pp.tile([128, D], f32, name=f"sin{g}")
                dma(t[:], rope_sin[g * 128:(g + 1) * 128, :])
                sin_g.append(t)
                t = pp.tile([128, D], f32, name=f"cos{g}")
                dma(t[:], rope_cos[g * 128:(g + 1) * 128, :])
                cos_g.append(t)
            n1g_b = bcast_row(pp, n1_g, D, "n1g_b")
            n1b_b = bcast_row(pp, n1_b, D, "n1b_b")

            for g in range(NG):
                mean = pp.tile([128, 1], f32, name=f"mean{g}")
                m2 = pp.tile([128, 1], f32, name=f"m2ln{g}")
                tmp = pp.tile([128, D], f32, name=f"lntmp{g}")
                nc.vector.tensor_reduce(mean[:], xg[g][:], AxX, Alu.add)
                nc.vector.tensor_scalar(mean[:], mean[:], 1.0 / D, None, Alu.mult)
                nc.vector.tensor_scalar(tmp[:], xg[g][:], mean[:], None, Alu.subtract)
                nc.vector.scalar_tensor_tensor(tmp[:], tmp[:], 1.0, tmp[:], Alu.mult,
                                               Alu.mult, accum_out=m2[:])
                nc.vector.tensor_scalar(m2[:], m2[:], 1.0 / D, 1e-5, Alu.mult, Alu.add)
                rstd = pp.tile([128, 1], f32, name=f"rstd{g}")
                nc.scalar.activation(rstd[:], m2[:], Act.Sqrt)
                nc.vector.reciprocal(rstd[:], rstd[:])
                nc.vector.tensor_scalar(xn[g][:], xg[g][:], mean[:], rstd[:],
                                        Alu.subtract, Alu.mult)
                nc.vector.scalar_tensor_tensor(xn[g][:], xn[g][:], 1.0, n1g_b[:],
                                               Alu.mult, Alu.mult)
                nc.vector.tensor_tensor(xn[g][:], xn[g][:], n1b_b[:], Alu.add)
                t_xr = pp.tile([128, D], f32, name=f"xr{g}")
                rot = pp.tile([128, D], f32, name=f"rot{g}")
                ev = lambda a: a.rearrange("p (a two) -> p a two", two=2)[:, :, 0]
                od = lambda a: a.rearrange("p (a two) -> p a two", two=2)[:, :, 1]
                nc.vector.tensor_scalar(ev(rot[:]), od(xn[g][:]), -1.0, None, Alu.mult)
                nc.vector.tensor_copy(od(rot[:]), ev(xn[g][:]))
                nc.vector.tensor_tensor(rot[:], rot[:], sin_g[g][:], Alu.mult)
                nc.vector.scalar_tensor_tensor(t_xr[:], xn[g][:], 1.0, cos_g[g][:],
                                               Alu.mult, Alu.mult)
                nc.vector.tensor_tensor(t_xr[:], t_xr[:], rot[:], Alu.add)
                xr.append(t_xr)

            # ctx = mean over tokens
            ctx_ps = pool_ps.tile([1, D], f32, name="ctx_ps", tag="Tps",
                                  padded_shape=[128, 512])
            for g in range(NG):
                nc.tensor.matmul(ctx_ps[:1, :], ones_sb[:], xr[g][:],
                                 start=(g == 0), stop=(g == NG - 1))
            ctx_row = pp.tile([1, D], f32, name="ctx_row")
            nc.vector.tensor_scalar(ctx_row[:], ctx_ps[:1, :], 1.0 / S, None, Alu.mult)

            xrT = pp.tile([128, 4 * S], f32, name="xrT")
            for g in range(NG):
                for kc in range(4):
                    transpose_to(xrT[:, kc * S + g * 128: kc * S + (g + 1) * 128],
                                 xr[g][:, kc * 128:(kc + 1) * 128], f"xrT{g}{kc}")
            ctxT = pp.tile([128, 4], f32, name="ctxT")
            for kc in range(4):
                transpose_to(ctxT[:, kc:kc + 1], ctx_row[:, kc * 128:(kc + 1) * 128],
                             f"ctxT{kc}")

            # --- merged int/sel/win MLP head: h1 = gelu(comb @ [w1i|w1s|w1w])
            # (int first so its 64-row lhsT slice sits at base partition 0;
            # sel's 32-row slice sits at base 64 -- both legal spans)
            H1 = 2 * P + 64 + 64          # int(64) + sel(32) + win(64)
            w1a = pp.tile([128, 4 * H1], f32, name="mh_w1a")
            w1b = pp.tile([128, 4 * H1], f32, name="mh_w1b")
            nc.vector.memset(w1a[:], 0.0)   # win sees only ctx -> zero xr-part
            for kc in range(4):
                dma(w1a[:, kc * H1: kc * H1 + 64],
                    int_w1[kc * 128:(kc + 1) * 128, :])
                dma(w1a[:, kc * H1 + 64: kc * H1 + 96],
                    sel_w1[kc * 128:(kc + 1) * 128, :])
                dma(w1b[:, kc * H1: kc * H1 + 64],
                    int_w1[D + kc * 128: D + (kc + 1) * 128, :])
                dma(w1b[:, kc * H1 + 64: kc * H1 + 96],
                    sel_w1[D + kc * 128: D + (kc + 1) * 128, :])
                dma(w1b[:, kc * H1 + 96: kc * H1 + 160],
                    win_w1[kc * 128:(kc + 1) * 128, :])
            b1_b = pp.tile([128, H1], f32, name="mh_b1b")
            dma(b1_b[:, 0:64], int_b1[:].to_broadcast([128, 64]))
            dma(b1_b[:, 64:96], sel_b1[:].to_broadcast([128, 32]))
            dma(b1_b[:, 96:160], win_b1[:].to_broadcast([128, 64]))
            selw2_sb = pp.tile([96, P], f32, name="selw2")
            dma(selw2_sb[64:96, :], sel_w2[:])
            selb2_b = bcast_row(pp, sel_b2, P, "selb2b")
            intw2_sb = pp.tile([64, 1], f32, name="intw2")
            dma(intw2_sb[:], int_w2[:])
            intb2_b = bcast_row(pp, int_b2, 1, "intb2b")
            winw2_sb = pp.tile([64, 1], f32, name="winw2")
            dma(winw2_sb[:], win_w2[:])
            winb2_b = bcast_row(pp, win_b2, 1, "winb2b")
            qp_b = bcast_row(pp, qpoly, 4, "qpb")
            v1_ps = pool_ps.tile([1, H1], f32, name="v1ps", tag="Tps",
                                 padded_shape=[128, 512])
            for kc in range(4):
                nc.tensor.matmul(v1_ps[:1, :], ctxT[:, kc:kc + 1],
                                 w1b[:, kc * H1:(kc + 1) * H1],
                                 start=(kc == 0), stop=(kc == 3))
            v1 = pp.tile([1, H1], f32, name="mh_v1")
            nc.vector.tensor_copy(v1[:], v1_ps[:1, :])
            v1_b = pp.tile([128, H1], f32, name="mh_v1b")
            pbcast(pp, v1_b[:], v1[:], H1, "mhv1")
            for g in range(NG):
                h1_ps = pool_ps.tile([128, H1], f32, name="h1ps", tag="Tps",
                                     padded_shape=[128, 512])
                for kc in range(4):
                    nc.tensor.matmul(
                        h1_ps[:], xrT[:, kc * S + g * 128: kc * S + (g + 1) * 128],
                        w1a[:, kc * H1:(kc + 1) * H1],
                        start=(kc == 0), stop=(kc == 3))
                h1 = pp.tile([128, H1], f32, name=f"mh_h1_{g}")
                nc.vector.tensor_tensor(h1[:], h1_ps[:], v1_b[:], Alu.add)
                nc.vector.tensor_tensor(h1[:], h1[:], b1_b[:], Alu.add)
                gelu_(pp, h1[:], f"mhg{g}")
                h1Ta = pp.tile([96, 128], f32, name=f"h1Ta_{g}")
                transpose_to(h1Ta[:], h1[:, 0:96], f"h1Ta{g}")
                # sel head -> softmax -> pw
                h2_ps = pool_ps.tile([128, P], f32, name="h2ps", tag="Tps",
                                     padded_shape=[128, 512])
                nc.tensor.matmul(h2_ps[:, :P], h1Ta[64:96, :], selw2_sb[64:96, :],
                                 start=True, stop=True)
                sel = pp.tile([128, P], f32, name=f"sel_{g}")
                nc.vector.tensor_tensor(sel[:], h2_ps[:, :P], selb2_b[:], Alu.add)
                mx = pp.tile([128, 1], f32, name=f"selmx{g}")
                nc.vector.tensor_reduce(mx[:], sel[:], AxX, Alu.max)
                nc.vector.tensor_scalar(sel[:], sel[:], mx[:], None, Alu.subtract)
                nc.scalar.activation(sel[:], sel[:], Act.Exp)
                sm = pp.tile([128, 1], f32, name=f"selsm{g}")
                nc.vector.tensor_reduce(sm[:], sel[:], AxX, Alu.add)
                rs = pp.tile([128, 1], f32, name=f"selrs{g}")
                nc.vector.reciprocal(rs[:], sm[:])
                nc.vector.tensor_scalar(pw_sb[g][:], sel[:], rs[:], None, Alu.mult)
                transpose_to(pwt[g][:], pw_sb[g][:], f"pwT{g}")
                nc.vector.tensor_copy(pwt_r[g][:], pwt[g][:])
                # bf16 hi/lo split of pw for the precise P4 matmul
                nc.vector.tensor_copy(pwt_hi[g][:], pwt[g][:])
                hi32 = pp.tile([P, 128], f32, name=f"hi32_{g}")
                nc.vector.tensor_copy(hi32[:], pwt_hi[g][:])
                nc.vector.tensor_tensor(hi32[:], pwt[g][:], hi32[:], Alu.subtract)
                nc.vector.tensor_copy(pwt_lo[g][:], hi32[:])
                # int head -> inten
                i2_ps = pool_ps.tile([128, 1], f32, name="i2ps", tag="Tps",
                                     padded_shape=[128, 512])
                nc.tensor.matmul(i2_ps[:, :1], h1Ta[0:64, :], intw2_sb[:],
                                 start=True, stop=True)
                ii = pp.tile([128, 1], f32, name=f"ii{g}")
                nc.vector.tensor_tensor(ii[:], i2_ps[:, :1], intb2_b[:], Alu.add)
                nc.scalar.activation(inten[g][:], ii[:], Act.Sigmoid)
                if DEBUG:
                    dma(dbg["dbg_pw"][g * 128:(g + 1) * 128, :], pw_sb[g][:])
                if g == 0:
                    # win head (token-independent; computed per-token on g0)
                    h1Tw = pp.tile([64, 128], f32, name="h1Tw")
                    transpose_to(h1Tw[:], h1[:, 96:160], "h1Tw")
                    wv_ps = pool_ps.tile([128, 1], f32, name="wvps", tag="Tps",
                                         padded_shape=[128, 512])
                    nc.tensor.matmul(wv_ps[:, :1], h1Tw[:], winw2_sb[:],
                                     start=True, stop=True)
                    winv = pp.tile([128, 1], f32, name="winv")
                    nc.vector.tensor_tensor(winv[:], wv_ps[:, :1], winb2_b[:],
                                            Alu.add)
                    nc.scalar.activation(winv[:], winv[:], Act.Sigmoid)
                    nc.vector.tensor_scalar(winv[:], winv[:],
                                            float(MAX_SEQ - 256), 256.0,
                                            Alu.mult, Alu.add)
                    nc.vector.tensor_scalar(kk_b[:], winv[:],
                                            0.1 / MAX_SEQ * DD, None, Alu.mult)
                    # floor() robust to the f32->i32 convert rounding mode
                    ki = pp.tile([128, 1], dt.int32, name="ki")
                    nc.vector.tensor_copy(ki[:], kk_b[:])
                    kf2 = pp.tile([128, 1], f32, name="kf2")
                    nc.vector.tensor_copy(kf2[:], ki[:])
                    kgt = pp.tile([128, 1], f32, name="kgt")
                    nc.vector.tensor_tensor(kgt[:], kf2[:], kk_b[:], Alu.is_gt)
                    nc.vector.tensor_tensor(kk_b[:], kf2[:], kgt[:], Alu.subtract)
                    nc.vector.tensor_scalar(kk_b[:], kk_b[:], 1.0, None, Alu.max)
                    u = pp.tile([128, 1], f32, name="qu")
                    nc.scalar.activation(u[:], kk_b[:], Act.Ln,
                                         scale=float(1.0 / DD))
                    nc.vector.tensor_scalar(zq_b[:], qp_b[:, 0:1], u[:],
                                            qp_b[:, 1:2], Alu.mult, Alu.add)
                    nc.vector.tensor_scalar(zq_b[:], zq_b[:], u[:], qp_b[:, 2:3],
                                            Alu.mult, Alu.add)
                    nc.vector.tensor_scalar(zq_b[:], zq_b[:], u[:], qp_b[:, 3:4],
                                            Alu.mult, Alu.add)
                    nc.scalar.activation(lkk_b[:], kk_b[:], Act.Ln)

            # sigma per token via Gram: q2 = pw^T G pw ; t0 = z*sqrt(q2/DD)*inten
            for g in range(NG):
                sig_ps = pool_ps.tile([128, P], f32, name="sigps", tag="Tps",
                                      padded_shape=[128, 128])
                nc.tensor.matmul(sig_ps[:], pwt[g][:], G_sb[:], start=True, stop=True)
                q2 = pp.tile([128, 1], f32, name=f"q2_{g}")
                scr = pp.tile([128, P], f32, name=f"q2scr{g}", tag="q2scr")
                nc.vector.scalar_tensor_tensor(scr[:], sig_ps[:], 1.0, pw_sb[g][:],
                                               Alu.mult, Alu.mult, accum_out=q2[:])
                sig = pp.tile([128, 1], f32, name=f"sig{g}")
                nc.scalar.activation(sig[:], q2[:], Act.Sqrt, scale=float(1.0 / DD))
                nc.vector.tensor_tensor(sig[:], sig[:], zq_b[:], Alu.mult)
                nc.vector.tensor_tensor(t0[g][:], sig[:], inten[g][:], Alu.mult)
                if DEBUG:
                    dma(dbg["dbg_t0"][g * 128:(g + 1) * 128, :], t0[g][:])

            if DEBUG:
                for g in range(NG):
                    dma(dbg["dbg_xn"][g * 128:(g + 1) * 128, :], xn[g][:])
                    dma(dbg["dbg_xr"][g * 128:(g + 1) * 128, :], xr[g][:])
                    dma(dbg["dbg_inten"][g * 128:(g + 1) * 128, :], inten[g][:])
                dma(dbg["dbg_scal"][:, 0:1], kk_b[0:1, 0:1])
                dma(dbg["dbg_scal"][:, 1:2], winv[0:1, 0:1])
                dma(dbg["dbg_scal"][:, 2:3], zq_b[0:1, 0:1])

        # =========== helpers: stream patterns & rematerialize F ===========
        # Both passes emit PAIRED chunks: one [128, 1024] 2-bank PSUM tile per
        # (c, c+1) so downstream scalar/vector ops amortize fixed costs.
        def flow_pass_r(g, consume, pat_pool):
            """fp32r pass (counting-grade precision)."""
            for w in range(16):
                patw = pat_pool.tile([P, 2048], f32r, name="patw", tag="patw",
                                     bufs=3)
                dma(patw[:], pat_r[:, w * 2048:(w + 1) * 2048])
                for m in range(0, 4, 2):
                    c = w * 4 + m
                    ps = pool_mm.tile([128, 1024], f32, name="Fps", tag="Fps")
                    for h in range(2):
                        nc.tensor.matmul(ps[:, h * 512:(h + 1) * 512],
                                         pwt_r[g][:],
                                         patw[:, (m + h) * 512:(m + h + 1) * 512],
                                         start=True, stop=True)
                    consume(c, ps)

        def flow_pass_hl(g, consume, pat_pool):
            """3-term bf16 split pass: hi*hi + hi*lo + lo*hi (~2^-16 precision,
            runs at full bf16 PE rate unlike fp32's half-rate 2-slice form)."""
            for w in range(16):
                pwh = pat_pool.tile([P, 2048], bf16, name="pwh", tag="pwh", bufs=3)
                pwl = pat_pool.tile([P, 2048], bf16, name="pwl", tag="pwl", bufs=3)
                dma(pwh[:], pat_hi[:, w * 2048:(w + 1) * 2048])
                dma(pwl[:], pat_lo[:, w * 2048:(w + 1) * 2048])
                for m in range(0, 4, 2):
                    c = w * 4 + m
                    ps = pool_mm.tile([128, 1024], f32, name="Fps", tag="Fps")
                    for h in range(2):
                        sl = slice((m + h) * 512, (m + h + 1) * 512)
                        po = ps[:, h * 512:(h + 1) * 512]
                        nc.tensor.matmul(po, pwt_hi[g][:], pwh[:, sl],
                                         start=True, stop=False)
                        nc.tensor.matmul(po, pwt_hi[g][:], pwl[:, sl],
                                         start=False, stop=False)
                        nc.tensor.matmul(po, pwt_lo[g][:], pwh[:, sl],
                                         start=False, stop=True)
                    consume(c, ps)

        # =============== ladder helpers ===============
        # g*(1-1.25*2^-11) lies 0.625..1.25 fp16-ULP below grid point g for any
        # mantissa, so RTN-to-fp16 lands exactly on the previous grid point.
        PREV16 = float(1.0 - 1.25 * 2.0 ** -11)

        def build_rungs(pool, center, scale_consts, g, name):
            """rungs at fp16-grid midpoints around center; returns (mids, lmids)"""
            nl = len(scale_consts)
            mids = pool.tile([128, nl], f32, name=f"{name}_mid{g}")
            lmids = pool.tile([128, nl], f32, name=f"{name}_lmid{g}")
            nmids = pool.tile([128, nl], f32, name=f"{name}_nmid{g}")
            graw = pool.tile([128, nl], f32, name=f"{name}_graw{g}")
            gf = pool.tile([128, nl], f32, name=f"{name}_gf{g}")
            g16 = pool.tile([128, nl], f16, name=f"{name}_g16{g}")
            gdec = pool.tile([128, nl], f16, name=f"{name}_gdec{g}")
            for j in range(nl):
                nc.vector.tensor_scalar(graw[:, j:j + 1], center[:],
                                        float(scale_consts[j]), None, Alu.mult)
            nc.vector.tensor_copy(g16[:], graw[:])              # rtn to fp16 grid
            nc.vector.tensor_copy(gf[:], g16[:])                # grid point, f32
            nc.vector.tensor_scalar(graw[:], gf[:], PREV16, None, Alu.mult)
            nc.vector.tensor_copy(gdec[:], graw[:])             # prev grid point
            nc.vector.tensor_copy(mids[:], gdec[:])
            nc.vector.tensor_tensor(mids[:], mids[:], gf[:], Alu.add)
            nc.vector.tensor_scalar(mids[:], mids[:], 0.5, None, Alu.mult)
            nc.vector.tensor_scalar(nmids[:], mids[:], -1.0, None, Alu.mult)
            nc.scalar.activation(lmids[:], mids[:], Act.Ln)
            return mids, lmids, nmids

        def count_rungs(pool, Ag, mids, nmids, nl, cl, scr_v, scr_s, g, name,
                        ns=2):
            """cl[:, j] = # (Ag >= mids[:, j]); quarters split scalar/vector
            (`ns` of 4 quarters on the scalar engine).

            Scalar quarters use Sign(A - mid) accumulated: S = #ge - #lt, so
            #ge = 0.5*S + QW/2 per quarter (mids sit strictly between fp16
            grid points, so A - mid never equals 0)."""
            ch = pool.tile([128, 5], f32, name=f"{name}_ch{g}", tag="cnt_ch")
            for j in range(nl):
                for q in range(4):
                    Aq = Ag[:, q * QW:(q + 1) * QW]
                    if q < ns:
                        nc.scalar.activation(scr_s[:], Aq, Act.Sign,
                                             bias=nmids[:, j:j + 1],
                                             accum_out=ch[:, q:q + 1])
                    else:
                        nc.vector.tensor_scalar(scr_v[:], Aq, mids[:, j:j + 1],
                                                None, Alu.is_ge, Alu.add,
                                                accum_out=ch[:, q:q + 1])
                if ns > 0:
                    nc.vector.tensor_reduce(cl[:, j:j + 1], ch[:, 0:ns], AxX,
                                            Alu.add)
                    nc.vector.tensor_scalar(cl[:, j:j + 1], cl[:, j:j + 1], 0.5,
                                            float(ns * QW // 2), Alu.mult, Alu.add)
                    nc.vector.tensor_reduce(ch[:, 4:5], ch[:, ns:4], AxX, Alu.add)
                    nc.vector.tensor_tensor(cl[:, j:j + 1], cl[:, j:j + 1],
                                            ch[:, 4:5], Alu.add)
                else:
                    nc.vector.tensor_reduce(cl[:, j:j + 1], ch[:, 0:4], AxX,
                                            Alu.add)

        def interp2_th(pool, cl, lmids, th_out, g, name):
            """log-log linear interp of count->kk over 2 rungs."""
            lc = pool.tile([128, 2], f32, name=f"{name}_lc{g}")
            nc.vector.tensor_scalar(lc[:], cl[:], 1.0, None, Alu.max)
            nc.scalar.activation(lc[:], lc[:], Act.Ln)
            num = pool.tile([128, 1], f32, name=f"{name}_num{g}")
            den = pool.tile([128, 1], f32, name=f"{name}_den{g}")
            dl = pool.tile([128, 1], f32, name=f"{name}_dl{g}")
            nc.vector.tensor_scalar(num[:], lc[:, 0:1], lkk_b[:], None,
                                    Alu.subtract)
            nc.vector.tensor_scalar(den[:], lc[:, 0:1], lc[:, 1:2], None,
                                    Alu.subtract)
            nc.vector.tensor_scalar(den[:], den[:], 1e-5, None, Alu.max)
            nc.vector.tensor_scalar(dl[:], lmids[:, 1:2], lmids[:, 0:1], None,
                                    Alu.subtract)
            nc.vector.reciprocal(den[:], den[:])
            nc.vector.tensor_tensor(num[:], num[:], den[:], Alu.mult)
            nc.vector.tensor_tensor(num[:], num[:], dl[:], Alu.mult)
            nc.vector.tensor_scalar(num[:], num[:], lmids[:, 0:1], None, Alu.add)
            nc.scalar.activation(th_out[:], num[:], Act.Exp)

        def interp_th(pool, cl, lmids, th_out, g, name):
            """log-log piecewise-linear interp of count->kk over 3 rungs."""
            lc = pool.tile([128, 3], f32, name=f"{name}_lc{g}")
            nc.vector.tensor_scalar(lc[:], cl[:], 1.0, None, Alu.max)
            nc.scalar.activation(lc[:], lc[:], Act.Ln)
            shi = pool.tile([128, 1], f32, name=f"{name}_shi{g}")
            nc.vector.tensor_scalar(shi[:], cl[:, 1:2], kk_b[:], None, Alu.is_ge)
            slo = pool.tile([128, 1], f32, name=f"{name}_slo{g}")
            nc.vector.tensor_scalar(slo[:], shi[:], -1.0, 1.0, Alu.mult, Alu.add)

            def blend(dst, a_hi, a_lo, tmp):
                nc.vector.tensor_tensor(dst, a_hi, shi[:], Alu.mult)
                nc.vector.tensor_tensor(tmp, a_lo, slo[:], Alu.mult)
                nc.vector.tensor_tensor(dst, dst, tmp, Alu.add)

            tmp = pool.tile([128, 1], f32, name=f"{name}_tmp{g}")
            num = pool.tile([128, 1], f32, name=f"{name}_num{g}")
            den = pool.tile([128, 1], f32, name=f"{name}_den{g}")
            base = pool.tile([128, 1], f32, name=f"{name}_base{g}")
            dl = pool.tile([128, 1], f32, name=f"{name}_dl{g}")
            d01 = pool.tile([128, 1], f32, name=f"{name}_d01{g}")
            d12 = pool.tile([128, 1], f32, name=f"{name}_d12{g}")
            # num = (lc[seg_lo_idx] - lkk)
            nc.vector.tensor_scalar(d01[:], lc[:, 1:2], lkk_b[:], None, Alu.subtract)
            nc.vector.tensor_scalar(d12[:], lc[:, 0:1], lkk_b[:], None, Alu.subtract)
            blend(num[:], d01[:], d12[:], tmp[:])
            # den = (lc[lo] - lc[hi])
            nc.vector.tensor_scalar(d01[:], lc[:, 1:2], lc[:, 2:3], None, Alu.subtract)
            nc.vector.tensor_scalar(d12[:], lc[:, 0:1], lc[:, 1:2], None, Alu.subtract)
            blend(den[:], d01[:], d12[:], tmp[:])
            nc.vector.tensor_scalar(den[:], den[:], 1e-5, None, Alu.max)
            # base / dl
            blend(base[:], lmids[:, 1:2], lmids[:, 0:1], tmp[:])
            nc.vector.tensor_scalar(d01[:], lmids[:, 2:3], lmids[:, 1:2], None,
                                    Alu.subtract)
            nc.vector.tensor_scalar(d12[:], lmids[:, 1:2], lmids[:, 0:1], None,
                                    Alu.subtract)
            blend(dl[:], d01[:], d12[:], tmp[:])
            nc.vector.reciprocal(den[:], den[:])
            nc.vector.tensor_tensor(num[:], num[:], den[:], Alu.mult)
            nc.vector.tensor_tensor(num[:], num[:], dl[:], Alu.mult)
            nc.vector.tensor_tensor(base[:], base[:], num[:], Alu.add)
            nc.scalar.activation(th_out[:], base[:], Act.Exp)

        # =============== P1: |F| -> fp16 + two-stage ladder ===============
        t_stage = pool_dram.tile([S, NL1], f32, name="t_stage")
        t_out = pool_dram.tile([S, NL1], f32, name="t_out", addr_space="Shared")
        t2_stage = pool_dram.tile([S, NL2], f32, name="t2_stage")
        t2_out = pool_dram.tile([S, NL2], f32, name="t2_out", addr_space="Shared")

        e1 = [float(np.exp(-DLT1)), float(np.exp(DLT1))]
        e2 = [float(np.exp(-DLT2)), 1.0, float(np.exp(DLT2))]

        with tc.tile_pool(name="selpool", bufs=1) as sp:
            A16 = sp.tile([128, NG * FREE], f16, name="A16")
            scr_v = sp.tile([128, QW], f16, name="scr_v")
            scr_s = sp.tile([128, QW], f16, name="scr_s")

            # stage-1 rungs are known before P1 (t0 from the Gram sigma), so
            # each group's stage-1 counts run right after that group's fill --
            # the scalar engine (in-order) must not queue all ABS first.
            lm1 = []
            rungs1 = [build_rungs(sp, t0[g], e1, g, "s1") for g in range(NG)]
            for g in range(NG):
                def consume_p1(c, ps, g=g):
                    nc.scalar.activation(
                        A16[:, g * FREE + c * 512: g * FREE + (c + 2) * 512],
                        ps[:], Act.Abs, scale=inten[g][:])
                flow_pass_r(g, consume_p1, sp)
                mids, lmids, nmids = rungs1[g]
                lm1.append(lmids)
                cl = sp.tile([128, NL1], f32, name=f"cl1_{g}")
                count_rungs(sp, A16[:, g * FREE:(g + 1) * FREE], mids, nmids,
                            NL1, cl, scr_v, scr_s, g, "s1", ns=1)
                dma(t_stage[g * 128:(g + 1) * 128, :], cl[:])
                if DEBUG:
                    dma(dbg["dbg_mid"][g * 128:(g + 1) * 128, 0:NL1], mids[:])
            nc.gpsimd.collective_compute(
                "AllReduce", Alu.add, replica_groups=RG,
                ins=[t_stage[:]], outs=[t_out[:]])
            for g in range(NG):
                cl = sp.tile([128, NL1], f32, name=f"cl1g_{g}")
                dma(cl[:], t_out[g * 128:(g + 1) * 128, :])
                if DEBUG:
                    dma(dbg["dbg_cnt"][g * 128:(g + 1) * 128, 0:NL1], cl[:])
                interp2_th(sp, cl, lm1[g], th1[g][:], g, "i1")

            # stage 2
            lm2 = []
            for g in range(NG):
                mids, lmids, nmids = build_rungs(sp, th1[g], e2, g, "s2")
                lm2.append(lmids)
                cl = sp.tile([128, NL2], f32, name=f"cl2_{g}")
                count_rungs(sp, A16[:, g * FREE:(g + 1) * FREE], mids, nmids,
                            NL2, cl, scr_v, scr_s, g, "s2", ns=2 + g)
                dma(t2_stage[g * 128:(g + 1) * 128, :], cl[:])
            nc.gpsimd.collective_compute(
                "AllReduce", Alu.add, replica_groups=RG,
                ins=[t2_stage[:]], outs=[t2_out[:]])
            for g in range(NG):
                cl = sp.tile([128, NL2], f32, name=f"cl2g_{g}")
                dma(cl[:], t2_out[g * 128:(g + 1) * 128, :])
                if DEBUG:
                    dma(dbg["dbg_cnt2"][g * 128:(g + 1) * 128, 0:NL2], cl[:])
                interp_th(sp, cl, lm2[g], th2[g][:], g, "i2")
                if DEBUG:
                    dma(dbg["dbg_th"][g * 128:(g + 1) * 128, 0:1], th1[g][:])
                    dma(dbg["dbg_th"][g * 128:(g + 1) * 128, 1:2], th2[g][:])

        # =============== P4: final masked matvec (split-bf16 pass) ===============
        fo_stage = pool_dram.tile([S, ISLICE], f32, name="fo_stage")
        fo_out = pool_dram.tile([NCORES, S, ISLICE], f32, name="fo_out",
                                addr_space="Shared")
        tailP = ctx.enter_context(tc.tile_pool(name="tailP", bufs=1))
        fo_full = [tailP.tile([128, D], f32, name=f"fo_full{g}") for g in range(NG)]

        # tail weights prefetch (emitted before P4: DMAs fill the SBUF space
        # A16 frees and overlap P4 compute; all tail matmuls run in fp16)
        tw = ctx.enter_context(tc.tile_pool(name="tailW", bufs=1))
        w1x = tw.tile([128, 4 * D], f16, name="w1x")
        w1m = tw.tile([128, 4 * D], f16, name="w1m")
        w2sb = tw.tile([128, 4 * D], f16, name="w2sb")
        upw = tw.tile([128, 4 * 8 * D], f16, name="upw")
        dnw = tw.tile([128, 16 * D], f16, name="dnw")
        for kc in range(4):
            dma(w1x[:, kc * D:(kc + 1) * D], mem_w1[kc * 128:(kc + 1) * 128, :])
            dma(w1m[:, kc * D:(kc + 1) * D],
                mem_w1[D + kc * 128: D + (kc + 1) * 128, :])
            dma(w2sb[:, kc * D:(kc + 1) * D], mem_w2[kc * 128:(kc + 1) * 128, :])
            dma(upw[:, kc * 8 * D:(kc + 1) * 8 * D],
                up_w[kc * 128:(kc + 1) * 128, :])
        for hc in range(16):
            dma(dnw[:, hc * D:(hc + 1) * D], down_w[hc * 128:(hc + 1) * 128, :])
        b2_b = bcast_row(tw, mem_b2, D, "b2b")
        b2T_sb = tw.tile([128, 4], f32, name="b2T_sb")
        dma(b2T_sb[:], mem_b2T[:])
        upbT_sb = tw.tile([128, 32], f32, name="upbT_sb")
        dma(upbT_sb[:], up_bT[:])
        dnb_b = bcast_row(tw, down_b, D, "dnb_b")
        n2g_b = bcast_row(tw, n2_g, D, "n2g_b")
        n2b_b = bcast_row(tw, n2_b, D, "n2b_b")
        b1row = tw.tile([1, D], f32, name="b1row")
        dma(b1row[:], mem_b1[:])
        memx = tw.tile([128, 4 * D], f32, name="memx")
        for kc in range(4):
            dma(memx[:, kc * D:(kc + 1) * D],
                memory_bank[kc * 128:(kc + 1) * 128, :])

        with tc.tile_pool(name="p4pool", bufs=1) as fp:
            XI16 = []
            for g in range(NG):
                t = fp.tile([128, D], f16, name=f"XI16_{g}")
                nc.vector.tensor_scalar(t[:], xn[g][:], inten[g][:], None, Alu.mult)
                XI16.append(t)
            for g in range(NG):
                FO = fp.tile([128, ISLICE], f32, name=f"FO{g}")

                def consume_p4(c, ps, g=g, FO=FO):
                    At = fp.tile([128, 1024], f32, name="At", tag="At", bufs=3)
                    FM = fp.tile([128, 1024], f16, name="FM", tag="FM", bufs=3)
                    sc16 = fp.tile([128, 1024], f16, name="sc16", tag="sc16",
                                   bufs=3)
                    nc.scalar.activation(At[:], ps[:], Act.Abs, scale=inten[g][:])
                    nc.vector.scalar_tensor_tensor(FM[:], At[:], th2[g][:], ps[:],
                                                   Alu.is_ge, Alu.mult)
                    for h in range(2):
                        nc.vector.scalar_tensor_tensor(
                            sc16[:, h * 512:(h + 1) * 512],
                            FM[:, h * 512:(h + 1) * 512], 1.0, XI16[g][:],
                            Alu.mult, Alu.mult,
                            accum_out=FO[:, c + h:c + h + 1])
                flow_pass_hl(g, consume_p4, fp)
                dma(fo_stage[g * 128:(g + 1) * 128, :], FO[:])

        nc.gpsimd.collective_compute(
            "AllGather", Alu.bypass, replica_groups=RG,
            ins=[fo_stage[:]], outs=[fo_out[:]])

        # close the flow PSUM pool so the tail pool fits the 8-bank budget
        _flowmm_cm.__exit__(None, None, None)

        # =============== tail (transposed layout, fp16 matmuls) ===============
        # All lhsT operands the tail needs are produced directly by matmuls in
        # transposed layout -- only co itself is PE-transposed (8 tiles).
        co = [tailP.tile([128, D], f32, name=f"co{g}") for g in range(NG)]
        with tc.tile_pool(name="psumTL", bufs=6, space="PSUM") as pool_tl, \
                tc.tile_pool(name="tailpool", bufs=1) as tp:
            for g in range(NG):
                for cidx in range(NCORES):
                    dma(fo_full[g][:, cidx * ISLICE:(cidx + 1) * ISLICE],
                        fo_out[cidx, g * 128:(g + 1) * 128, :])
                if DEBUG:
                    dma(dbg["dbg_fo"][g * 128:(g + 1) * 128, :], fo_full[g][:])
                nc.vector.tensor_tensor(co[g][:], xg[g][:], fo_full[g][:], Alu.add)
                mean = tp.tile([128, 1], f32, name=f"mean2{g}")
                m2 = tp.tile([128, 1], f32, name=f"m2ln2{g}")
                tmp = tp.tile([128, D], f32, name=f"ln2tmp{g}", tag="tmp")
                nc.vector.tensor_reduce(mean[:], co[g][:], AxX, Alu.add)
                nc.vector.tensor_scalar(mean[:], mean[:], 1.0 / D, None, Alu.mult)
                nc.vector.tensor_scalar(tmp[:], co[g][:], mean[:], None,
                                        Alu.subtract)
                nc.vector.scalar_tensor_tensor(tmp[:], tmp[:], 1.0, tmp[:],
                                               Alu.mult, Alu.mult, accum_out=m2[:])
                nc.vector.tensor_scalar(m2[:], m2[:], 1.0 / D, 1e-5, Alu.mult,
                                        Alu.add)
                rstd = tp.tile([128, 1], f32, name=f"rstd2{g}")
                nc.scalar.activation(rstd[:], m2[:], Act.Sqrt)
                nc.vector.reciprocal(rstd[:], rstd[:])
                nc.vector.tensor_scalar(co[g][:], co[g][:], mean[:], rstd[:],
                                        Alu.subtract, Alu.mult)
                nc.vector.scalar_tensor_tensor(co[g][:], co[g][:], 1.0, n2g_b[:],
                                               Alu.mult, Alu.mult)
                nc.vector.tensor_tensor(co[g][:], co[g][:], n2b_b[:], Alu.add)

            # coT (fp16) [128, 4*S]: chunk kc holds co^T[kc*128:(kc+1)*128, :]
            coT = tp.tile([128, 4 * S], f16, name="coT")
            for g in range(NG):
                for kc in range(4):
                    transpose_to(coT[:, kc * S + g * 128: kc * S + (g + 1) * 128],
                                 co[g][:, kc * 128:(kc + 1) * 128], f"coT{g}{kc}")

            # memory-bank mean -> memvT; cvec = memv @ W1m + b1 (const row)
            mem_ps = pool_ps.tile([1, D], f32, name="memps", tag="Tps",
                                  padded_shape=[128, 512])
            for kc in range(4):
                nc.tensor.matmul(mem_ps[:1, :], ones_sb[:],
                                 memx[:, kc * D:(kc + 1) * D],
                                 start=(kc == 0), stop=(kc == 3))
            memv = tp.tile([1, D], f32, name="memv")
            nc.vector.tensor_scalar(memv[:], mem_ps[:1, :], 1.0 / 512.0, None,
                                    Alu.mult)
            memvT = tp.tile([128, 4], f16, name="memvT")
            for kc in range(4):
                transpose_to(memvT[:, kc:kc + 1], memv[:, kc * 128:(kc + 1) * 128],
                             f"memvT{kc}")
            cps = pool_ps.tile([1, D], f32, name="cps", tag="Tps",
                               padded_shape=[128, 512])
            for kc in range(4):
                nc.tensor.matmul(cps[:1, :], memvT[:, kc:kc + 1],
                                 w1m[:, kc * D:(kc + 1) * D],
                                 start=(kc == 0), stop=(kc == 3))
            cvec = tp.tile([1, D], f32, name="cvec")
            nc.vector.tensor_tensor(cvec[:], cps[:1, :], b1row[:], Alu.add)
            cvecT = tp.tile([128, 4], f32, name="cvecT")
            for kc in range(4):
                transpose_to(cvecT[:, kc:kc + 1], cvec[:, kc * 128:(kc + 1) * 128],
                             f"cvT{kc}")

            # mhT[h, s] = silu(co @ W1x + cvec)^T : 4 h-chunks of [128, 256]
            mhT = tp.tile([128, 4 * S], f16, name="mhT")
            for hc in range(4):
                ps = pool_tl.tile([128, 512], f32, name="tl_ps", tag="tlps")
                for kc in range(4):
                    nc.tensor.matmul(
                        ps[:, :S],
                        w1x[:, kc * D + hc * 128: kc * D + hc * 128 + 128],
                        coT[:, kc * S:(kc + 1) * S],
                        start=(kc == 0), stop=(kc == 3))
                pre = tp.tile([128, S], f32, name="mhpre", tag="mhpre", bufs=2)
                nc.vector.tensor_scalar(pre[:], ps[:, :S], cvecT[:, hc:hc + 1],
                                        None, Alu.add)
                sg = tp.tile([128, S], f32, name="mhsg", tag="mhsg", bufs=2)
                nc.scalar.activation(sg[:], pre[:], Act.Sigmoid)
                nc.vector.tensor_tensor(mhT[:, hc * S:(hc + 1) * S], pre[:],
                                        sg[:], Alu.mult)

            # mo (token layout) -> co2 = co + mo + b2
            for g in range(NG):
                ps = pool_tl.tile([128, 512], f32, name="tl_ps", tag="tlps")
                for hc in range(4):
                    nc.tensor.matmul(
                        ps[:], mhT[:, hc * S + g * 128: hc * S + g * 128 + 128],
                        w2sb[:, hc * D:(hc + 1) * D],
                        start=(hc == 0), stop=(hc == 3))
                nc.vector.scalar_tensor_tensor(co[g][:], ps[:], 1.0, co[g][:],
                                               Alu.mult, Alu.add)
                nc.vector.tensor_tensor(co[g][:], co[g][:], b2_b[:], Alu.add)
            # moT -> co2T = coT + moT + b2T (dc-chunks of [128, 256])
            co2T = tp.tile([128, 4 * S], f16, name="co2T")
            for dc in range(4):
                ps = pool_tl.tile([128, 512], f32, name="tl_ps", tag="tlps")
                for hc in range(4):
                    nc.tensor.matmul(
                        ps[:, :S],
                        w2sb[:, hc * D + dc * 128: hc * D + dc * 128 + 128],
                        mhT[:, hc * S:(hc + 1) * S],
                        start=(hc == 0), stop=(hc == 3))
                pre = tp.tile([128, S], f32, name="moTpre", tag="mhpre", bufs=2)
                nc.vector.tensor_scalar(pre[:], ps[:, :S], b2T_sb[:, dc:dc + 1],
                                        None, Alu.add)
                nc.vector.tensor_tensor(co2T[:, dc * S:(dc + 1) * S], pre[:],
                                        coT[:, dc * S:(dc + 1) * S], Alu.add)

            # ffT gate chunks -> silu(gate); val chunks fused multiply -> gvT
            silg = tp.tile([128, 16 * S], f16, name="silg")
            for hc in range(16):
                ps = pool_tl.tile([128, 512], f32, name="tl_ps", tag="tlps")
                for kc in range(4):
                    nc.tensor.matmul(
                        ps[:, :S],
                        upw[:, kc * 8 * D + hc * 128: kc * 8 * D + hc * 128 + 128],
                        co2T[:, kc * S:(kc + 1) * S],
                        start=(kc == 0), stop=(kc == 3))
                gt = tp.tile([128, S], f32, name="gt", tag="gt", bufs=2)
                nc.vector.tensor_scalar(gt[:], ps[:, :S], upbT_sb[:, hc:hc + 1],
                                        None, Alu.add)
                sg2 = tp.tile([128, S], f32, name="sg2", tag="sg2", bufs=2)
                nc.scalar.activation(sg2[:], gt[:], Act.Sigmoid)
                nc.vector.tensor_tensor(silg[:, hc * S:(hc + 1) * S], gt[:],
                                        sg2[:], Alu.mult)
            gvT = tp.tile([128, 16 * S], f16, name="gvT")
            for hc in range(16):
                ps = pool_tl.tile([128, 512], f32, name="tl_ps", tag="tlps")
                for kc in range(4):
                    nc.tensor.matmul(
                        ps[:, :S],
                        upw[:, kc * 8 * D + (16 + hc) * 128:
                            kc * 8 * D + (16 + hc) * 128 + 128],
                        co2T[:, kc * S:(kc + 1) * S],
                        start=(kc == 0), stop=(kc == 3))
                nc.vector.scalar_tensor_tensor(
                    gvT[:, hc * S:(hc + 1) * S], ps[:, :S],
                    upbT_sb[:, 16 + hc:17 + hc], silg[:, hc * S:(hc + 1) * S],
                    Alu.add, Alu.mult)

            # ffn = gv @ down_w + down_b ; out = co2 + ffn
            for g in range(NG):
                ps = pool_tl.tile([128, 512], f32, name="tl_ps", tag="tlps")
                for hc in range(16):
                    nc.tensor.matmul(
                        ps[:], gvT[:, hc * S + g * 128: hc * S + g * 128 + 128],
                        dnw[:, hc * D:(hc + 1) * D],
                        start=(hc == 0), stop=(hc == 15))
                ffn = tp.tile([128, D], f32, name=f"ffn{g}")
                nc.vector.scalar_tensor_tensor(ffn[:], ps[:], 1.0, co[g][:],
                                               Alu.mult, Alu.add)
                nc.vector.tensor_tensor(ffn[:], ffn[:], dnb_b[:], Alu.add)
                dma(out_dram[g * 128:(g + 1) * 128, :], ffn[:])

    return nc


def _install_ntff_shim():
    """Reconstitute the missing antenv.axon_hooks module so
    run_bass_kernel_spmd(trace=True) can reach the axon NTFF profiler."""
    import sys
    import types

    if "antenv.axon_hooks" in sys.modules:
        return
    import antenv

    mod = types.ModuleType("antenv.axon_hooks")
    _h = [None]
    mod.set_axon_ntff_profile_hook = lambda h: _h.__setitem__(0, h)
    mod.get_axon_ntff_profile_hook = lambda: _h[0]
    sys.modules["antenv.axon_hooks"] = mod
    antenv.axon_hooks = mod
    try:
        from trn_agent_boot.trn_boot import _ntff_profile_via_ctypes

        mod.set_axon_ntff_profile_hook(
            _ntff_profile_via_ctypes("/opt/axon/libaxon_pjrt.so"))
    except Exception:
        pass


def kernel(**inputs):
    from concourse.bass_utils import run_bass_kernel_spmd
    _install_ntff_shim()

    sin, cos, qpoly = _host_constants()
    x = np.ascontiguousarray(np.asarray(inputs["x"], np.float32).reshape(S, D))
    patterns = np.ascontiguousarray(np.asarray(inputs["flow_patterns"], np.float32))

    nc = build_kernel()
    nc.finalize()

    def a(k):
        return np.ascontiguousarray(np.asarray(inputs[k], np.float32))

    def row(k):
        return np.ascontiguousarray(np.asarray(inputs[k], np.float32).reshape(1, -1))

    base = {
        "x": x,
        "sel_w1": a("sel_w1"), "sel_b1": row("sel_b1"),
        "sel_w2": a("sel_w2"), "sel_b2": row("sel_b2"),
        "win_w1": a("win_w1"), "win_b1": row("win_b1"),
        "win_w2": a("win_w2"), "win_b2": row("win_b2"),
        "int_w1": a("int_w1"), "int_b1": row("int_b1"),
        "int_w2": a("int_w2"), "int_b2": row("int_b2"),
        "mem_w1": a("mem_w1").astype(np.float16), "mem_b1": row("mem_b1"),
        "mem_w2": a("mem_w2").astype(np.float16), "mem_b2": row("mem_b2"),
        "mem_b2T": np.ascontiguousarray(
            np.asarray(inputs["mem_b2"], np.float32).reshape(4, 128).T),
        "memory_bank": a("memory_bank"),
        "up_w": a("up_w").astype(np.float16), "up_b": row("up_b"),
        "up_bT": np.ascontiguousarray(
            np.asarray(inputs["up_b"], np.float32).reshape(32, 128).T),
        "down_w": a("down_w").astype(np.float16), "down_b": row("down_b"),
        "n1_g": row("n1_g"), "n1_b": row("n1_b"),
        "n2_g": row("n2_g"), "n2_b": row("n2_b"),
        "rope_sin": sin, "rope_cos": cos,
        "qpoly": qpoly.reshape(1, 4),
    }
    import ml_dtypes
    in_maps = []
    for c in range(NCORES):
        m = dict(base)
        psl = np.ascontiguousarray(
            patterns[:, c * ISLICE:(c + 1) * ISLICE, :].reshape(P, FREE))
        m["pat_r"] = psl
        phi = psl.astype(ml_dtypes.bfloat16)
        m["pat_hi"] = phi
        m["pat_lo"] = (psl - phi.astype(np.float32)).astype(ml_dtypes.bfloat16)
        # [FREE, P] -> [128, (FREE/128)*P]: partition p holds rows p, p+128, ...
        m["pat_T"] = np.ascontiguousarray(
            psl.T.reshape(FREE // 128, 128, P).transpose(1, 0, 2).reshape(
                128, (FREE // 128) * P))
        in_maps.append(m)

    trace = os.environ.get("KERNEL_TRACE", "0") == "1"
    res = run_bass_kernel_spmd(nc, in_maps, list(range(NCORES)), trace=trace)
    out0 = res.results[0]
    kernel.last_results = res.results
    kernel.last_exec_ns = getattr(res, "exec_time_ns", None)
    return out0["out"].reshape(B, S, D).astype(np.float32)


if __name__ == "__main__":
    data = np.load("/tmp/inputs.npz")
    inputs = {k: data[k] for k in data.files}
    out = kernel(**inputs)
    print("out", out.shape, float(np.abs(out).max()))
